# revision 30
# baseline (speedup 1.0000x reference)
"""DNeRF renderer on 8 Trainium2 cores (Bass/Tile) — v2.

Data-parallel over rays (1024 rays/core, 8 ray-tiles of 128 rays).

v2 structure (vs v1):
- All big matmuls in float32r (1 cyc/row on the PE, full-rate fp32).
- Two-phase schedule: phase 1 = coarse MLP + sigma for all 8 ray-tiles,
  then the per-ray sort/searchsorted machinery BATCHED over pairs of
  ray-tiles (segmented scans via (mult,add)-reset tricks), then phase 2 =
  fine MLP + compositing per ray-tile.  PE streams matmuls back-to-back
  while DVE/Act/Pool run the machinery of earlier tiles.
- Sigma eviction packed into [128,512] psum via tile_position 32-blocks
  (free-size-bound engine cost drops 4x).
- Layer-1 bias folded into the relu eviction (per-partition bias column),
  dropping the ones-row from the matmul rhs.
- bc2 bias folded into the rgb eviction copies.
- dterm broadcast-add runs on the (otherwise idle) Pool engine.
"""

import numpy as np
import ml_dtypes
from contextlib import ExitStack

import concourse.bass as bass
import concourse.bacc as bacc
import concourse.mybir as mybir
import concourse.tile as tile
from concourse.bass_utils import run_bass_kernel_spmd
from concourse import library_config

dt = mybir.dt
Alu = mybir.AluOpType
Act = mybir.ActivationFunctionType
AxX = mybir.AxisListType.X

NCORES = 8
NRAYS = 8192
R = NRAYS // NCORES      # rays per core
P = 128                  # rays per tile (partitions)
T = R // P               # ray-tiles per core
S = 64                   # num_steps
U = 64                   # upsample_steps
M = S + U                # merged samples
MB = 2                   # ray-tiles per machinery batch
NG = T // MB
MIN_NEAR = 0.05
M24 = 16777216.0         # 2^24

_BUILT = None
SIM_SAFE = False


def _build():
    nc = bacc.Bacc("TRN2", target_bir_lowering=False, debug=False,
                   num_devices=NCORES)

    def din(name, shape, dtype=dt.float32):
        return nc.dram_tensor(name, shape, dtype, kind="ExternalInput").ap()

    rays_o = din("rays_o_k", [P, T, 3])
    rays_d = din("rays_d_k", [P, T, 3])
    dT_in = din("dT_k", [T, 4, P])
    v128 = din("v128", [P, S])
    cc = din("cc", [P, 8])
    segmaskM = din("segmaskM", [P, T * M])
    iop1T = din("iop1T", [P, T * M])
    seg256T = din("seg256T", [P, T * M])
    segmask62 = din("segmask62", [P, T * 62])
    iev62T = din("iev62T", [P, T * 62])
    oneSst = din("oneSst", [P, T * S])
    zeroM = din("zeroM", [P, M])
    lhsT6 = din("lhsT6", [6, 128], dt.float32r)
    b1col = din("b1col", [128, 1])
    w0pair = din("w0pair", [128, 2], dt.bfloat16)
    wgcpair = din("wgcpair", [128, 128], dt.bfloat16)
    wc2pair = din("wc2pair", [128, 6], dt.float32r)
    dlhs = din("dlhs", [4, 64])
    bc2col = din("bc2col", [6, 1])
    bgrep = din("bgrep", [P, 3])
    scl_in = din("scl", [P, 4])

    img_out = nc.dram_tensor("img_k", [P, T, 3], dt.float32,
                             kind="ExternalOutput").ap()


    def dep0(ap_):
        # partition-strided APs confuse subtile dep tracking; anchor the
        # tracked range at offset 0 so read/write overlap is detected.
        return bass.AP(tensor=ap_.tensor, offset=ap_.offset, ap=ap_.ap,
                       dep_tracking_offset=0)

    def mmr(out, lhsT, rhs, **kw):
        nc.tensor.matmul(out, lhsT.bitcast(dt.float32r),
                         rhs.bitcast(dt.float32r), **kw)

    with tile.TileContext(nc) as tc, ExitStack() as ctx:

        cpool = ctx.enter_context(tc.tile_pool(name="consts", bufs=1))
        spool = ctx.enter_context(tc.tile_pool(name="setup", bufs=1))
        mpool = ctx.enter_context(tc.tile_pool(name="mach", bufs=1))
        wpool = ctx.enter_context(tc.tile_pool(name="work", bufs=2))
        rpool = ctx.enter_context(tc.tile_pool(name="rhs", bufs=1))
        bpool = ctx.enter_context(tc.tile_pool(name="big", bufs=2))
        pp1 = ctx.enter_context(tc.tile_pool(name="ps1", bufs=1, space="PSUM"))
        pp2 = ctx.enter_context(tc.tile_pool(name="ps2", bufs=2, space="PSUM"))
        dpool = ctx.enter_context(tc.tile_pool(name="dram", bufs=2, space="DRAM"))

        def cload(ap_in, shape, tag, dtype=dt.float32):
            t_ = cpool.tile(shape, dtype, tag=tag, name=tag)
            nc.sync.dma_start(t_[:], ap_in)
            return t_

        v128_s = cload(v128, [P, S], tag='c_v128')
        cc_s = cload(cc, [P, 8], tag='c_cc')
        smM_s = cload(segmaskM, [P, T * M], tag='c_smM')
        iop1_s = cload(iop1T, [P, T * M], tag='c_iop1')
        s256_s = cload(seg256T, [P, T * M], tag='c_s256')
        sm62_s = cload(segmask62, [P, T * 62], tag='c_sm62')
        iev_s = cload(iev62T, [P, T * 62], tag='c_iev')
        oneS_s = cload(oneSst, [P, T * S], tag='c_oneS')
        zeroM_s = cload(zeroM, [P, M], tag='c_zeroM')
        lhsT6_s = cload(lhsT6, [6, 128], tag='c_lhsT6', dtype=dt.float32r)
        b1_s = cload(b1col, [128, 1], tag='c_b1col')
        w0p_s = cload(w0pair, [128, 2], tag='c_w0pair', dtype=dt.bfloat16)
        wgc_s = cload(wgcpair, [128, 128], tag='c_wgc', dtype=dt.bfloat16)
        wc2_s = cload(wc2pair, [128, 6], tag='c_wc2', dtype=dt.float32r)
        dlhs_s = cload(dlhs, [4, 64], tag='c_dlhs')
        bc2_s = cload(bc2col, [6, 1], tag='c_bc2col')
        bg_s = cload(bgrep, [P, 3], tag='c_bgrep')
        scl_s = cload(scl_in, [P, 4], tag='c_scl')
        ro_s = cload(rays_o, [P, T, 3], tag='c_rays_o')
        rd_s = cload(rays_d, [P, T, 3], tag='c_rays_d')

        ones_c = cc_s[:, 0:1]
        eps_c = cc_s[:, 1:2]
        e15_c = cc_s[:, 2:3]
        e5_c = cc_s[:, 3:4]
        nhalf_c = cc_s[:, 4:5]
        m24_c = cc_s[:, 5:6]
        nm24_c = cc_s[:, 6:7]
        n8003_c = cc_s[:, 7:8]
        bd2_0c = scl_s[:, 0:1]

        def bc(col, n):
            return col.broadcast_to((P, n))

        # ============ STAGE A: ray setup (batched over T) ============
        n24 = T * 3

        def st(shape, tag, dtype=dt.float32):
            return spool.tile(shape, dtype, tag=tag, name=tag)

        negd = st([P, T, 3], 's_negd')
        nc.vector.tensor_scalar(negd[:], rd_s[:], -1.0, None, Alu.mult)
        absd = st([P, T, 3], 's_absd')
        nc.vector.tensor_tensor(absd[:], rd_s[:], negd[:], Alu.max)
        dmask = st([P, T, 3], 's_dmask', dt.uint8)
        nc.vector.tensor_scalar(dmask[:], absd[:], 1e-9, None, Alu.is_lt)
        dsafe = st([P, T, 3], 's_dsafe')
        nc.vector.select(dsafe[:].rearrange("p t c -> p (t c)"),
                         dmask[:].rearrange("p t c -> p (t c)"),
                         bc(eps_c, n24),
                         rd_s[:].rearrange("p t c -> p (t c)"))
        invd = st([P, T, 3], 's_invd')
        nc.vector.reciprocal(invd[:], dsafe[:])
        a1 = st([P, T, 3], 's_a1')
        nc.vector.scalar_tensor_tensor(a1[:], ro_s[:], 1.0, invd[:],
                                       Alu.add, Alu.mult)
        b1 = st([P, T, 3], 's_b1')
        nc.vector.scalar_tensor_tensor(b1[:], ro_s[:], -1.0, invd[:],
                                       Alu.add, Alu.mult)
        mx = st([P, T, 3], 's_mx')
        nc.vector.tensor_tensor(mx[:], a1[:], b1[:], Alu.max)
        mn = st([P, T, 3], 's_mn')
        nc.vector.tensor_tensor(mn[:], a1[:], b1[:], Alu.min)
        tmin = st([P, T], 's_tmin')
        nc.vector.tensor_reduce(tmin[:], mx[:], AxX, Alu.min)
        tmax = st([P, T], 's_tmax')
        nc.vector.tensor_reduce(tmax[:], mn[:], AxX, Alu.max)
        near = st([P, T], 's_near')
        nc.vector.tensor_scalar(near[:], tmin[:], -1.0, MIN_NEAR,
                                Alu.mult, Alu.max)
        tmaxt = st([P, T], 's_tmaxt')
        nc.vector.tensor_scalar(tmaxt[:], tmax[:], -1.0, None, Alu.mult)
        fmask = st([P, T], 's_fmask', dt.uint8)
        nc.vector.tensor_tensor(fmask[:], tmaxt[:], near[:], Alu.is_lt)
        nearp = st([P, T], 's_nearp')
        nc.vector.tensor_scalar(nearp[:], near[:], 1e-2, None, Alu.add)
        far = st([P, T], 's_far')
        nc.vector.select(far[:], fmask[:], nearp[:], tmaxt[:])
        rng = st([P, T], 's_rng')
        nc.vector.tensor_tensor(rng[:], far[:], near[:], Alu.subtract)
        dzv = st([P, T], 's_dzv')
        nc.vector.tensor_scalar(dzv[:], rng[:], 1.0 / 63.0, None, Alu.mult)
        sdv = st([P, T], 's_sdv')
        nc.vector.tensor_scalar(sdv[:], rng[:], 1.0 / 64.0, None, Alu.mult)
        invdz = st([P, T], 's_invdz')
        nc.vector.reciprocal(invdz[:], dzv[:])
        inv2dz = st([P, T], 's_inv2dz')
        nc.vector.tensor_scalar(inv2dz[:], invdz[:], 2.0, None, Alu.mult)
        mid0 = st([P, T], 's_mid0')
        nc.vector.scalar_tensor_tensor(mid0[:], dzv[:], 0.5, near[:],
                                       Alu.mult, Alu.add)
        aoff = st([P, T], 's_aoff')
        nc.vector.tensor_tensor(aoff[:], near[:], dzv[:], Alu.subtract)

        # coarse z grid, batched: zc3[p,t,s] = near[p,t] + v128[s]*rng[p,t]
        zc3 = st([P, T, S], 's_zc3')
        v3 = v128_s[:].rearrange("p (o s) -> p o s", o=1).broadcast_to((P, T, S))
        rng3 = rng[:].rearrange("p (t o) -> p t o", o=1).broadcast_to((P, T, S))
        near3 = near[:].rearrange("p (t o) -> p t o", o=1).broadcast_to((P, T, S))
        nc.vector.tensor_tensor(zc3[:], v3, rng3, Alu.mult)
        nc.vector.tensor_tensor(zc3[:], zc3[:], near3, Alu.add)

        # persistent cross-phase arrays
        h20T = st([P, T, S], 's_h20T')
        Z3 = st([P, T, M], 's_Z3')
        dl3 = st([P, T, M], 's_dl3')

        def b3(col2, n):
            # [P, MB] -> [P, MB, n] broadcast
            return col2.rearrange("p (t o) -> p t o", o=1) \
                .broadcast_to((P, MB, n))

        # ================= PHASE 1: coarse MLP + sigma =================
        for t in range(T):
            xyzc = wpool.tile([P, 3, S], dt.float32r, tag="xyzc", name="xyzc")
            for c in range(3):
                nc.vector.scalar_tensor_tensor(
                    xyzc[:, c, :], zc3[:, t, :], rd_s[:, t, c:c + 1],
                    bc(ro_s[:, t, c:c + 1], S), Alu.mult, Alu.add)
            scr = dpool.tile([3, P, S], dt.float32r, tag="xyzscr", name="xyzscr")
            nc.sync.dma_start(scr[:].rearrange("c p s -> p c s"), xyzc[:])
            rhs6 = rpool.tile([6, P * S // 2], dt.float32r, tag="rhs6c",
                              name="rhs6c")
            scrf = scr[:].rearrange("c p s -> c (p s)")
            half = P * S // 2
            nc.sync.dma_start(rhs6[0:3, :], scrf[:, 0:half])
            nc.sync.dma_start(rhs6[3:6, :], scrf[:, half:2 * half])

            sgs = dpool.tile([2, 64, S], dt.float32, tag="sigscr", name="sigscr")
            sgsf = sgs[:].rearrange("h p s -> (h p s)")

            pS = None
            for hf in range(4):          # 4 half-groups of 1024 cols
                pA = pp1.tile([128, 1024], dt.float32, tag="pA", name="pA")
                for c2 in range(2):
                    mmr(pA[:, 512 * c2:512 * (c2 + 1)], lhsT6_s[:],
                        rhs6[:, 1024 * hf + 512 * c2:1024 * hf + 512 * (c2 + 1)],
                        start=True, stop=True)
                rh1 = bpool.tile([128, 1024], dt.bfloat16, tag="rh1", name="rh1")
                if hf % 2 == 0:
                    nc.vector.tensor_scalar(rh1[:], pA[:], b1_s[:], 0.0,
                                            Alu.add, Alu.max)
                else:
                    nc.scalar.activation(rh1[:], pA[:], Act.Relu, bias=b1_s[:])
                if hf % 2 == 0:
                    pS = pp1.tile([128, 512], dt.float32, tag="pS", name="pS")
                    if SIM_SAFE:
                        nc.vector.memset(pS[:], 0.0)
                for c2 in range(2):
                    cc_g = 2 * (hf % 2) + c2
                    pos = 32 * cc_g
                    nc.tensor.matmul(pS[pos:pos + 2, :], w0p_s[:],
                                     rh1[:, 512 * c2:512 * (c2 + 1)],
                                     start=True, stop=True,
                                     tile_position=(0, pos))
                if hf % 2 == 1:
                    ps_i = hf // 2
                    sg_sb = bpool.tile([128, 512], dt.float32, tag="sgsb",
                                       name="sgsb")
                    if ps_i == 0:
                        nc.vector.tensor_copy(sg_sb[:], pS[:])
                    else:
                        nc.scalar.copy(sg_sb[:], pS[:])
                    # rows (32a+h) -> dram (h, p=32*ps+8a+q, s)
                    for a_ in range(4):
                        dst = sgs[:, 32 * ps_i + 8 * a_:
                                  32 * ps_i + 8 * (a_ + 1), :] \
                            .rearrange("h p s -> h (p s)")
                        nc.sync.dma_start(dst, sg_sb[32 * a_:32 * a_ + 2, :])
            nc.sync.dma_start(h20T[:, t, :],
                              sgs[:].rearrange("h p s -> (h p) s"))

        # ================= machinery (batched per MB tiles) =================
        def mt(shape, tag, dtype=dt.float32):
            return mpool.tile(shape, dtype, tag=tag, name=tag)

        for mb in range(NG):
            t0 = mb * MB
            colM = slice(t0 * M, (t0 + MB) * M)
            colS = slice(t0 * S, (t0 + MB) * S)
            col62 = slice(t0 * 62, (t0 + MB) * 62)
            h20v = h20T[:, t0:t0 + MB, :]
            dz_sl = dzv[:, t0:t0 + MB]
            sd_sl = sdv[:, t0:t0 + MB]
            near_sl = near[:, t0:t0 + MB]
            i2dz_sl = inv2dz[:, t0:t0 + MB]
            mid0_sl = mid0[:, t0:t0 + MB]
            aoff_sl = aoff[:, t0:t0 + MB]

            TM2 = MB * M

            # --- coarse composite weights ---
            sig3 = mt([P, MB, S], 'm_sig3')
            nc.scalar.activation(sig3[:].rearrange("p t s -> p (t s)"),
                                 h20v.rearrange("p t s -> p (t s)"),
                                 Act.Exp, bias=bd2_0c)
            dsgc = mt([P, MB, S], 'm_dsgc')
            nc.vector.tensor_tensor(dsgc[:], sig3[:], b3(dz_sl, S), Alu.mult)
            nc.vector.tensor_tensor(dsgc[:, :, S - 1:S], sig3[:, :, S - 1:S],
                                    b3(sd_sl, 1), Alu.mult)
            emc = mt([P, MB, S], 'm_emc')
            nc.scalar.activation(emc[:].rearrange("p t s -> p (t s)"),
                                 dsgc[:].rearrange("p t s -> p (t s)"),
                                 Act.Exp, scale=-1.0)
            d0c = mt([P, MB, S], 'm_d0c')
            nc.vector.memset(d0c[:, :, 0:1], 0.0)
            nc.scalar.activation(d0c[:, :, 1:S], emc[:, :, 0:S - 1],
                                 Act.Identity, bias=e15_c)
            Tc = mt([P, MB, S], 'm_Tc')
            nc.vector.tensor_tensor_scan(
                Tc[:].rearrange("p t s -> p (t s)"),
                d0c[:].rearrange("p t s -> p (t s)"),
                oneS_s[:, colS], 0.0, Alu.mult, Alu.add)
            alpha = mt([P, MB, S], 'm_alpha')
            nc.scalar.activation(alpha[:].rearrange("p t s -> p (t s)"),
                                 emc[:].rearrange("p t s -> p (t s)"),
                                 Act.Identity, scale=-1.0, bias=ones_c)
            wts = mt([P, MB, S], 'm_wts')
            nc.vector.tensor_tensor(wts[:], alpha[:], Tc[:], Alu.mult)

            # --- pdf/cdf over weights[:,1:63] ---
            wp = mt([P, MB, 62], 'm_wp')
            nc.scalar.activation(wp[:], wts[:, :, 1:63], Act.Identity,
                                 bias=e5_c)
            ssum = mt([P, MB], 'm_ssum')
            nc.vector.tensor_reduce(ssum[:], wp[:], AxX, Alu.add)
            pinv = mt([P, MB], 'm_pinv')
            nc.vector.reciprocal(pinv[:], ssum[:])
            pdf = mt([P, MB, 62], 'm_pdf')
            nc.vector.tensor_tensor(pdf[:], wp[:], b3(pinv[:], 62), Alu.mult)
            cdf = mt([P, MB, 62], 'm_cdf')
            nc.vector.tensor_tensor_scan(
                cdf[:].rearrange("p t s -> p (t s)"), sm62_s[:, col62],
                pdf[:].rearrange("p t s -> p (t s)"), 0.0, Alu.mult, Alu.add)

            # --- scatter cdf onto per-segment 128-slot timelines ---
            r2 = mt([P, MB, 62], 'm_r2')
            r2f = r2[:].rearrange("p t s -> p (t s)")
            cdff = cdf[:].rearrange("p t s -> p (t s)")
            nc.scalar.activation(r2f, cdff, Act.Identity, scale=128.0,
                                 bias=m24_c)
            nc.scalar.activation(r2f, r2f, Act.Identity, bias=nm24_c)
            idx2f = mt([P, MB, 124], 'm_idx2f')
            i4 = idx2f[:].rearrange("p t (a b) -> p t a b", b=2)
            ev = i4[:, :, :, 0:1].rearrange("p t a b -> p t (a b)")
            od = i4[:, :, :, 1:2].rearrange("p t a b -> p t (a b)")
            nc.vector.tensor_tensor(
                ev, r2[:], iev_s[:, col62].rearrange("p (t s) -> p t s", t=MB),
                Alu.add)
            nc.scalar.activation(od, ev, Act.Identity, bias=ones_c)
            idx2i = mt([P, MB * 124], 'm_idx2i', dt.int16)
            nc.scalar.copy(idx2i[:], idx2f[:].rearrange("p t s -> p (t s)"))
            tlc2 = mt([P, MB * 256], 'm_tlc2', dt.int16)
            nc.gpsimd.local_scatter(tlc2[:], cdff.bitcast(dt.int16),
                                    idx2i[:], channels=P,
                                    num_elems=MB * 256, num_idxs=MB * 124)
            tlc = tlc2[:].bitcast(dt.float32)
            tlc3 = tlc.rearrange("p (t s) -> p t s", t=MB)

            # --- fills and counts on the timeline ---
            notC = mt([P, MB, M], 'm_notC')
            notCf = notC[:].rearrange("p t s -> p (t s)")
            nc.vector.tensor_scalar(notCf, tlc, 0.0, None, Alu.is_equal)
            notCp = mt([P, MB * M], 'm_notCp')
            nc.gpsimd.tensor_tensor(notCp[:], notCf, smM_s[:, colM], Alu.mult)
            kp1 = mt([P, MB, M], 'm_kp1')
            kp1f = kp1[:].rearrange("p t s -> p (t s)")
            nc.vector.tensor_tensor_scan(kp1f, smM_s[:, colM], notCf,
                                         0.0, Alu.mult, Alu.add)
            uu = mt([P, MB * M], 'm_uu')
            nc.scalar.activation(uu[:], kp1f, Act.Identity,
                                 scale=1.0 / 64.0, bias=nhalf_c)
            cntC = mt([P, MB * M], 'm_cntC')
            nc.vector.tensor_tensor(cntC[:], iop1_s[:, colM], kp1f,
                                    Alu.subtract)
            ffwd = mt([P, MB * M], 'm_ffwd')
            nc.vector.tensor_tensor_scan(ffwd[:], notCp[:], tlc, 0.0,
                                         Alu.mult, Alu.add)
            rnotC = mt([P, MB, M], 'm_rnotC')
            nc.scalar.copy(rnotC[:], notC[:, :, ::-1])
            rnotCp = mt([P, MB * M], 'm_rnotCp')
            nc.gpsimd.tensor_tensor(rnotCp[:],
                                    rnotC[:].rearrange("p t s -> p (t s)"),
                                    smM_s[:, colM], Alu.mult)
            rtlc = mt([P, MB, M], 'm_rtlc')
            nc.scalar.copy(rtlc[:], tlc3[:, :, ::-1])
            rbwd = mt([P, MB, M], 'm_rbwd')
            nc.vector.tensor_tensor_scan(
                rbwd[:].rearrange("p t s -> p (t s)"), rnotCp[:],
                rtlc[:].rearrange("p t s -> p (t s)"), 0.0, Alu.mult, Alu.add)

            # --- inverse-CDF lerp at u slots ---
            den = mt([P, MB, M], 'm_den')
            nc.vector.tensor_tensor(den[:], rbwd[:, :, ::-1],
                                    ffwd[:].rearrange("p (t s) -> p t s", t=MB),
                                    Alu.subtract)
            denf = den[:].rearrange("p t s -> p (t s)")
            mkd = mt([P, MB * M], 'm_mkd', dt.uint8)
            nc.vector.tensor_scalar(mkd[:], denf, 1e-5, None, Alu.is_lt)
            nc.vector.select(denf, mkd[:], bc(ones_c, TM2), denf)
            rden = mt([P, MB * M], 'm_rden')
            nc.vector.reciprocal(rden[:], denf)
            ttv = mt([P, MB * M], 'm_ttv')
            nc.vector.tensor_tensor(ttv[:], uu[:], ffwd[:], Alu.subtract)
            nc.vector.tensor_tensor(ttv[:], ttv[:], rden[:], Alu.mult)
            bg0 = mt([P, MB, M], 'm_bg0')
            nc.gpsimd.tensor_tensor(bg0[:],
                                    cntC[:].rearrange("p (t s) -> p t s", t=MB),
                                    b3(dz_sl, M), Alu.mult)
            nc.gpsimd.tensor_tensor(bg0[:], bg0[:], b3(mid0_sl, M), Alu.add)
            nz = mt([P, MB, M], 'm_nz')
            nc.vector.tensor_tensor(nz[:],
                                    ttv[:].rearrange("p (t s) -> p t s", t=MB),
                                    b3(dz_sl, M), Alu.mult)
            nc.vector.tensor_tensor(nz[:], nz[:], bg0[:], Alu.add)
            nzf = nz[:].rearrange("p t s -> p (t s)")

            # --- merge ranks into final (coarse ∪ fine) timeline ---
            q2 = mt([P, MB, M], 'm_q2')
            nc.vector.tensor_tensor(q2[:], nz[:], b3(near_sl, M), Alu.subtract)
            nc.vector.tensor_tensor(q2[:], q2[:], b3(i2dz_sl, M), Alu.mult)
            q2f = q2[:].rearrange("p t s -> p (t s)")
            nc.vector.tensor_scalar(q2f, q2f, 1.0, M24, Alu.add, Alu.add)
            nc.vector.tensor_scalar(q2f, q2f, M24, 0.0, Alu.subtract, Alu.max)
            nc.vector.tensor_scalar(q2f, q2f, 126.0, 8001.0, Alu.min, Alu.add)
            tk2 = mt([P, MB * M], 'm_tk2')
            nc.scalar.activation(tk2[:], kp1f, Act.Identity, scale=2.0,
                                 bias=n8003_c)
            mk2 = mt([P, MB * M], 'm_mk2')
            nc.vector.tensor_scalar(mk2[:], kp1f, 64.5, None, Alu.is_gt)
            minv = mt([P, MB * M], 'm_minv')
            nc.vector.scalar_tensor_tensor(minv[:], mk2[:], 1.0, notCf,
                                           Alu.add, Alu.subtract)
            m2 = mt([P, MB * M], 'm_m2')
            nc.vector.scalar_tensor_tensor(m2[:], minv[:], -4000.0, q2f,
                                           Alu.mult, Alu.add)
            ms = mt([P, MB * M], 'm_ms')
            nc.vector.tensor_tensor_scan(ms[:], smM_s[:, colM], m2[:],
                                         0.0, Alu.mult, Alu.max)
            rkv = mt([P, MB * M], 'm_rkv')
            nc.vector.tensor_tensor(rkv[:], tk2[:], ms[:], Alu.add)
            nc.vector.tensor_scalar(rkv[:], rkv[:], 254.0, None, Alu.min)
            nc.vector.scalar_tensor_tensor(rkv[:], minv[:], -4000.0, rkv[:],
                                           Alu.mult, Alu.add)
            fidx2f = mt([P, MB, 256], 'm_fidx2f')
            f4 = fidx2f[:].rearrange("p t (a b) -> p t a b", b=2)
            fev = f4[:, :, :, 0:1].rearrange("p t a b -> p t (a b)")
            fod = f4[:, :, :, 1:2].rearrange("p t a b -> p t (a b)")
            nc.vector.tensor_tensor(
                fev, rkv[:].rearrange("p (t s) -> p t s", t=MB),
                s256_s[:, colM].rearrange("p (t s) -> p t s", t=MB), Alu.add)
            nc.scalar.activation(fod, fev, Act.Identity, bias=ones_c)
            fidx2i = mt([P, MB * 256], 'm_fidx2i', dt.int16)
            nc.scalar.copy(fidx2i[:], fidx2f[:].rearrange("p t s -> p (t s)"))
            zf2 = mt([P, MB * 256], 'm_zf2', dt.int16)
            nc.gpsimd.local_scatter(zf2[:], nzf.bitcast(dt.int16),
                                    fidx2i[:], channels=P,
                                    num_elems=MB * 256, num_idxs=MB * 256)
            zsc = zf2[:].bitcast(dt.float32)

            # --- fill coarse slots with uniform grid ---
            isCC = mt([P, MB, M], 'm_isCC')
            isCCf = isCC[:].rearrange("p t s -> p (t s)")
            nc.vector.tensor_scalar(isCCf, zsc, 0.0, None, Alu.is_equal)
            cum2 = mt([P, MB, M], 'm_cum2')
            nc.vector.tensor_tensor_scan(
                cum2[:].rearrange("p t s -> p (t s)"), smM_s[:, colM],
                isCCf, 0.0, Alu.mult, Alu.add)
            zcf = mt([P, MB, M], 'm_zcf')
            nc.gpsimd.tensor_tensor(zcf[:], cum2[:], b3(dz_sl, M), Alu.mult)
            nc.gpsimd.tensor_tensor(zcf[:], zcf[:], b3(aoff_sl, M), Alu.add)
            Zv = Z3[:, t0:t0 + MB, :]
            nc.vector.tensor_tensor(Zv, isCC[:], zcf[:], Alu.mult)
            nc.vector.tensor_tensor(
                Zv, Zv, zsc.rearrange("p (t s) -> p t s", t=MB), Alu.add)
            dv = dl3[:, t0:t0 + MB, :]
            nc.vector.tensor_tensor(dv[:, :, 0:M - 1], Zv[:, :, 1:M],
                                    Zv[:, :, 0:M - 1], Alu.subtract)
            nc.scalar.copy(dv[:, :, M - 1:M],
                           sd_sl.rearrange("p (t o) -> p t o", o=1))

        # ================= PHASE 2: fine MLP + composite =================
        fh = P * M // 2
        for t in range(T):
            xyzm = wpool.tile([P, 3, M], dt.float32r, tag="xyzm", name="xyzm")
            for c in range(3):
                nc.vector.scalar_tensor_tensor(
                    xyzm[:, c, :], Z3[:, t, :], rd_s[:, t, c:c + 1],
                    bc(ro_s[:, t, c:c + 1], M), Alu.mult, Alu.add)
            scr2 = dpool.tile([3, P, M], dt.float32r, tag="xyzscr2",
                              name="xyzscr2")
            nc.sync.dma_start(scr2[:].rearrange("c p s -> p c s"), xyzm[:])
            dT_sb = wpool.tile([4, P], dt.float32, tag="dTsb", name="dTsb")
            nc.sync.dma_start(dT_sb[:], dT_in[t])
            rhs6f = rpool.tile([6, fh], dt.float32r, tag="rhs6f", name="rhs6f")
            scr2f = scr2[:].rearrange("c p s -> c (p s)")
            nc.sync.dma_start(rhs6f[0:3, :], scr2f[:, 0:fh])
            nc.sync.dma_start(rhs6f[3:6, :], scr2f[:, fh:2 * fh])

            # dterm pairs [128, 64] (v1 form)
            pD = pp1.tile([128, 512], dt.float32, tag="pS", name="pS")
            if SIM_SAFE:
                nc.vector.memset(pD[:], 0.0)
            nc.tensor.matmul(pD[0:64, 0:64], dlhs_s[:], dT_sb[:, 0:64],
                             start=True, stop=True)
            nc.tensor.matmul(pD[64:128, 0:64], dlhs_s[:], dT_sb[:, 64:128],
                             start=True, stop=True, tile_position=(0, 64))
            dtm = bpool.tile([128, 64], dt.float32, tag="dtm", name="dtm")
            nc.scalar.copy(dtm[:], pD[:, 0:64])

            sgs2 = dpool.tile([2, 64, M], dt.float32, tag="sigscr2",
                              name="sigscr2")
            sgs2f = sgs2[:].rearrange("h p s -> (h p s)")
            rgbs = dpool.tile([2, 3, 64, M], dt.float32, tag="rgbscr",
                              name="rgbscr")
            rgbsw = rgbs[:].rearrange("h c p s -> (h c) (p s)")

            for g in range(4):
                rgb_sb = bpool.tile([6, 2048], dt.float32, tag="rgbsb",
                                    name="rgbsb")
                pS = pp1.tile([128, 512], dt.float32, tag="pS", name="pS")
                if SIM_SAFE:
                    nc.vector.memset(pS[:], 0.0)
                for hf in range(2):
                    base = g * 2048 + hf * 1024
                    pA = pp1.tile([128, 1024], dt.float32, tag="pA", name="pA")
                    for c2 in range(2):
                        mmr(pA[:, 512 * c2:512 * (c2 + 1)], lhsT6_s[:],
                            rhs6f[:, base + 512 * c2:base + 512 * (c2 + 1)],
                            start=True, stop=True)
                    rh1f = bpool.tile([128, 1024], dt.bfloat16, tag="rh1",
                                      name="rh1")
                    if hf == 0:
                        nc.vector.tensor_scalar(rh1f[:], pA[:], b1_s[:], 0.0,
                                                Alu.add, Alu.max)
                    else:
                        nc.scalar.activation(rh1f[:], pA[:], Act.Relu,
                                             bias=b1_s[:])
                    for c2 in range(2):
                        cc4 = 2 * hf + c2
                        pos = 32 * cc4
                        nc.tensor.matmul(pS[pos:pos + 2, :], w0p_s[:],
                                         rh1f[:, 512 * c2:512 * (c2 + 1)],
                                         start=True, stop=True,
                                         tile_position=(0, pos))
                    pG = pp2.tile([128, 1024], dt.float32, tag="pG", name="pG")
                    for c2 in range(2):
                        nc.tensor.matmul(pG[:, 512 * c2:512 * (c2 + 1)],
                                         wgc_s[:],
                                         rh1f[:, 512 * c2:512 * (c2 + 1)],
                                         start=True, stop=True)
                    dsl = dtm[:, g * 16 + hf * 8:g * 16 + (hf + 1) * 8]
                    nc.vector.tensor_tensor(
                        pG[:].rearrange("h (a s) -> h a s", s=M),
                        pG[:].rearrange("h (a s) -> h a s", s=M),
                        dsl.rearrange("h (a o) -> h a o", o=1)
                        .broadcast_to((128, 8, M)), Alu.add)
                    ch1 = bpool.tile([128, 1024], dt.float32r, tag="ch1",
                                     name="ch1")
                    if hf == 0:
                        nc.scalar.activation(ch1[:], pG[:], Act.Relu)
                    else:
                        nc.vector.tensor_scalar(ch1[:], pG[:], 0.0, None,
                                                Alu.max)
                    for c2 in range(2):
                        pC = pp1.tile([6, 512], dt.float32, tag="pC", name="pC")
                        mmr(pC[:], wc2_s[:], ch1[:, 512 * c2:512 * (c2 + 1)],
                            start=True, stop=True)
                        osl = rgb_sb[:, 1024 * hf + 512 * c2:
                                     1024 * hf + 512 * (c2 + 1)]
                        if (2 * hf + c2) % 2 == 0:
                            nc.scalar.activation(osl, pC[:], Act.Identity,
                                                 bias=bc2_s[:])
                        else:
                            nc.vector.tensor_scalar(osl, pC[:], bc2_s[:],
                                                    None, Alu.add)
                # sigma eviction (packed) + DMA
                sg_sb = bpool.tile([128, 512], dt.float32, tag="sgsb",
                                   name="sgsb")
                if g % 2 == 0:
                    nc.vector.tensor_copy(sg_sb[:], pS[:])
                else:
                    nc.scalar.copy(sg_sb[:], pS[:])
                for a_ in range(4):
                    dst = sgs2[:, 16 * g + 4 * a_:16 * g + 4 * (a_ + 1), :] \
                        .rearrange("h p s -> h (p s)")
                    nc.sync.dma_start(dst, sg_sb[32 * a_:32 * a_ + 2, :])
                nc.sync.dma_start(rgbsw[:, g * 2048:(g + 1) * 2048], rgb_sb[:])

            # ---- composite in rays layout ----
            def wt(shape, tag, dtype=dt.float32):
                return wpool.tile(shape, dtype, tag=tag, name=tag)

            h20m = wt([P, M], "h20m")
            nc.sync.dma_start(h20m[:], sgs2[:].rearrange("h p s -> (h p) s"))
            sigm = wt([P, M], "sigm")
            nc.scalar.activation(sigm[:], h20m[:], Act.Exp, bias=bd2_0c)
            dsg2 = wt([P, M], "dsg2")
            nc.vector.tensor_tensor(dsg2[:], dl3[:, t, :], sigm[:], Alu.mult)
            em2 = wt([P, M], "em2")
            nc.scalar.activation(em2[:], dsg2[:], Act.Exp, scale=-1.0)
            sb2 = wt([P, M], "sb2")
            nc.vector.memset(sb2[:, 0:1], 1.0)
            nc.scalar.activation(sb2[:, 1:M], em2[:, 0:M - 1],
                                 Act.Identity, bias=e15_c)
            Tm = wt([P, M], "Tm")
            nc.vector.tensor_tensor_scan(Tm[:], sb2[:], zeroM_s[:], 1.0,
                                         Alu.mult, Alu.add)
            alpm = wt([P, M], "alpm")
            nc.scalar.activation(alpm[:], em2[:], Act.Identity,
                                 scale=-1.0, bias=ones_c)
            wm = wt([P, M], "wm")
            nc.vector.tensor_tensor(wm[:], alpm[:], Tm[:], Alu.mult)
            wsum = wt([P, 1], "wsum")
            nc.vector.tensor_reduce(wsum[:], wm[:], AxX, Alu.add)
            wmm = wt([P, M], "wmm")
            nc.vector.scalar_tensor_tensor(wmm[:], wm[:], 1e-4, wm[:],
                                           Alu.is_gt, Alu.mult)

            rgbp = wt([P, 3, M], "rgbp")
            for h_ in range(2):
                nc.sync.dma_start(rgbp[64 * h_:64 * (h_ + 1), :, :],
                                  rgbs[h_].rearrange("c p s -> p c s"))
            erg = wt([P, 3, M], "erg")
            nc.scalar.activation(erg[:].rearrange("p c s -> p (c s)"),
                                 rgbp[:].rearrange("p c s -> p (c s)"),
                                 Act.Exp, scale=-1.0)
            nc.scalar.activation(erg[:].rearrange("p c s -> p (c s)"),
                                 erg[:].rearrange("p c s -> p (c s)"),
                                 Act.Identity, bias=ones_c)
            rgbv = wt([P, 3, M], "rgbv")
            nc.vector.reciprocal(rgbv[:], erg[:])
            nc.vector.tensor_tensor(
                rgbv[:], rgbv[:],
                wmm[:].rearrange("p (o s) -> p o s", o=1)
                .broadcast_to((P, 3, M)), Alu.mult)
            img = wt([P, 3], "img")
            nc.vector.tensor_reduce(img[:], rgbv[:], AxX, Alu.add)
            bgw = wt([P, 1], "bgw")
            nc.vector.tensor_scalar(bgw[:], wsum[:], -1.0, 1.0, Alu.mult,
                                    Alu.add)
            nc.vector.scalar_tensor_tensor(img[:], bg_s[:], bgw[:], img[:],
                                           Alu.mult, Alu.add)
            nc.sync.dma_start(img_out[:, t, :], img[:])

    nc.compile()
    return nc


def _host_constants(inputs):
    Wd1 = np.asarray(inputs["Wd1"], np.float32)
    bd1 = np.asarray(inputs["bd1"], np.float32)
    Wd2 = np.asarray(inputs["Wd2"], np.float32)
    bd2 = np.asarray(inputs["bd2"], np.float32)
    Wc1 = np.asarray(inputs["Wc1"], np.float32)
    bc1 = np.asarray(inputs["bc1"], np.float32)
    Wc2 = np.asarray(inputs["Wc2"], np.float32)
    bc2 = np.asarray(inputs["bc2"], np.float32)
    tval = float(np.asarray(inputs["time"]).reshape(()))

    W1 = Wd1[:3]
    b1p = bd1 + tval * Wd1[3]
    w0 = Wd2[:, 0:1]
    Wgc = (Wd2[:, 1:].astype(np.float64) @ Wc1[3:].astype(np.float64)) \
        .astype(np.float32)
    bgc = (bd2[1:].astype(np.float64) @ Wc1[3:].astype(np.float64)) \
        .astype(np.float32)
    bd2_0 = float(bd2[0])

    lhsT6 = np.zeros((6, 128), np.float32)
    lhsT6[0:3, 0:64] = W1
    lhsT6[3:6, 64:128] = W1
    b1col = np.concatenate([b1p, b1p]).reshape(128, 1).astype(np.float32)

    w0pair = np.zeros((128, 2), np.float32)
    w0pair[0:64, 0:1] = w0
    w0pair[64:128, 1:2] = w0

    wgcpair = np.zeros((128, 128), np.float32)
    wgcpair[0:64, 0:64] = Wgc
    wgcpair[64:128, 64:128] = Wgc

    wc2pair = np.zeros((128, 6), np.float32)
    wc2pair[0:64, 0:3] = Wc2
    wc2pair[64:128, 3:6] = Wc2

    dlhs = np.zeros((4, 64), np.float32)
    dlhs[0:3] = Wc1[:3]
    dlhs[3] = bc1 + bgc

    bc2col = np.concatenate([bc2, bc2]).reshape(6, 1).astype(np.float32)


    v = np.linspace(0.0, 1.0, S, dtype=np.float32)

    jM = np.arange(T * M)
    j62 = np.arange(T * 62)
    jS = np.arange(T * S)
    segmaskM = (jM % M != 0).astype(np.float32)
    iop1T = (jM % M + 1).astype(np.float32)
    seg256T = (256 * ((jM // M) % MB)).astype(np.float32)
    segmask62 = (j62 % 62 != 0).astype(np.float32)
    iev62T = (2 * (j62 % 62) + 256 * ((j62 // 62) % MB)).astype(np.float32)
    oneSst = (jS % S == 0).astype(np.float32)

    def rep(row):
        return np.broadcast_to(row, (P,) + row.shape).copy()

    return {
        "v128": rep(v),
        "cc": rep(np.array([1.0, 1e-9, 1e-15, 1e-5, -1.0 / 128.0,
                    16777216.0, -16777216.0, -8003.0],
                   np.float32)),
        "segmaskM": rep(segmaskM),
        "iop1T": rep(iop1T),
        "seg256T": rep(seg256T),
        "segmask62": rep(segmask62),
        "iev62T": rep(iev62T),
        "oneSst": rep(oneSst),
        "zeroM": np.zeros((P, M), np.float32),
        "lhsT6": lhsT6, "b1col": b1col,
        "w0pair": w0pair.astype(ml_dtypes.bfloat16),
        "wgcpair": wgcpair.astype(ml_dtypes.bfloat16),
        "wc2pair": wc2pair, "dlhs": dlhs,
        "bc2col": bc2col,
        "bgrep": np.broadcast_to(
            np.asarray(inputs["background_color"], np.float32), (P, 3)).copy(),
        "scl": np.broadcast_to(
            np.array([bd2_0, 0, 0, 0], np.float32), (P, 4)).copy(),
    }


def kernel(**inputs):
    global _BUILT
    assert int(inputs["num_steps"]) == S
    assert int(inputs["upsample_steps"]) == U

    if _BUILT is None:
        _BUILT = _build()
    nc = _BUILT

    consts = _host_constants(inputs)
    ro = np.asarray(inputs["rays_o"], np.float32).reshape(NRAYS, 3)
    rd = np.asarray(inputs["rays_d"], np.float32).reshape(NRAYS, 3)

    in_maps = []
    for c in range(NCORES):
        sl_o = ro[c * R:(c + 1) * R].reshape(T, P, 3)
        sl_d = rd[c * R:(c + 1) * R].reshape(T, P, 3)
        dT = np.ones((T, 4, P), np.float32)
        dT[:, 0:3, :] = sl_d.transpose(0, 2, 1)
        m = {
            "rays_o_k": np.ascontiguousarray(sl_o.transpose(1, 0, 2)),
            "rays_d_k": np.ascontiguousarray(sl_d.transpose(1, 0, 2)),
            "dT_k": dT,
        }
        m.update(consts)
        in_maps.append(m)

    res = run_bass_kernel_spmd(nc, in_maps, core_ids=list(range(NCORES)))
    global LAST_RESULT
    LAST_RESULT = res
    outs = []
    for c in range(NCORES):
        img = res.results[c]["img_k"]
        outs.append(img.transpose(1, 0, 2).reshape(R, 3))
    return np.concatenate(outs, 0).reshape(1, NRAYS, 3)


# revision 31
# speedup vs baseline: 1.0115x; 1.0115x over previous
"""DNeRF renderer on 8 Trainium2 cores (Bass/Tile) — v2.

Data-parallel over rays (1024 rays/core, 8 ray-tiles of 128 rays).

v2 structure (vs v1):
- All big matmuls in float32r (1 cyc/row on the PE, full-rate fp32).
- Two-phase schedule: phase 1 = coarse MLP + sigma for all 8 ray-tiles,
  then the per-ray sort/searchsorted machinery BATCHED over pairs of
  ray-tiles (segmented scans via (mult,add)-reset tricks), then phase 2 =
  fine MLP + compositing per ray-tile.  PE streams matmuls back-to-back
  while DVE/Act/Pool run the machinery of earlier tiles.
- Sigma eviction packed into [128,512] psum via tile_position 32-blocks
  (free-size-bound engine cost drops 4x).
- Layer-1 bias folded into the relu eviction (per-partition bias column),
  dropping the ones-row from the matmul rhs.
- bc2 bias folded into the rgb eviction copies.
- dterm broadcast-add runs on the (otherwise idle) Pool engine.
"""

import numpy as np
import ml_dtypes
from contextlib import ExitStack

import concourse.bass as bass
import concourse.bacc as bacc
import concourse.mybir as mybir
import concourse.tile as tile
from concourse.bass_utils import run_bass_kernel_spmd
from concourse import library_config

dt = mybir.dt
Alu = mybir.AluOpType
Act = mybir.ActivationFunctionType
AxX = mybir.AxisListType.X

NCORES = 8
NRAYS = 8192
R = NRAYS // NCORES      # rays per core
P = 128                  # rays per tile (partitions)
T = R // P               # ray-tiles per core
S = 64                   # num_steps
U = 64                   # upsample_steps
M = S + U                # merged samples
MB = 2                   # ray-tiles per machinery batch
NG = T // MB
MIN_NEAR = 0.05
M24 = 16777216.0         # 2^24

_BUILT = None
SIM_SAFE = False


def _build():
    nc = bacc.Bacc("TRN2", target_bir_lowering=False, debug=False,
                   num_devices=NCORES)

    def din(name, shape, dtype=dt.float32):
        return nc.dram_tensor(name, shape, dtype, kind="ExternalInput").ap()

    rays_o = din("rays_o_k", [P, T, 3])
    rays_d = din("rays_d_k", [P, T, 3])
    dT_in = din("dT_k", [T, 4, P])
    v128 = din("v128", [P, S])
    cc = din("cc", [P, 8])
    segmaskM = din("segmaskM", [P, T * M])
    iop1T = din("iop1T", [P, T * M])
    seg256T = din("seg256T", [P, T * M])
    segmask62 = din("segmask62", [P, T * 62])
    iev62T = din("iev62T", [P, T * 62])
    oneSst = din("oneSst", [P, T * S])
    zeroM = din("zeroM", [P, M])
    lhsT6 = din("lhsT6", [6, 128], dt.float32r)
    b1col = din("b1col", [128, 1])
    w0pair = din("w0pair", [128, 2], dt.bfloat16)
    wgcpair = din("wgcpair", [128, 128], dt.bfloat16)
    wc2pair = din("wc2pair", [128, 6], dt.float32r)
    dlhs = din("dlhs", [4, 64])
    bc2col = din("bc2col", [6, 1])
    bgrep = din("bgrep", [P, 3])
    scl_in = din("scl", [P, 4])

    img_out = nc.dram_tensor("img_k", [P, T, 3], dt.float32,
                             kind="ExternalOutput").ap()


    def dep0(ap_):
        # partition-strided APs confuse subtile dep tracking; anchor the
        # tracked range at offset 0 so read/write overlap is detected.
        return bass.AP(tensor=ap_.tensor, offset=ap_.offset, ap=ap_.ap,
                       dep_tracking_offset=0)

    def mmr(out, lhsT, rhs, **kw):
        nc.tensor.matmul(out, lhsT.bitcast(dt.float32r),
                         rhs.bitcast(dt.float32r), **kw)

    with tile.TileContext(nc) as tc, ExitStack() as ctx:

        cpool = ctx.enter_context(tc.tile_pool(name="consts", bufs=1))
        spool = ctx.enter_context(tc.tile_pool(name="setup", bufs=1))
        mpool = ctx.enter_context(tc.tile_pool(name="mach", bufs=1))
        wpool = ctx.enter_context(tc.tile_pool(name="work", bufs=2))
        rpool = ctx.enter_context(tc.tile_pool(name="rhs", bufs=1))
        bpool = ctx.enter_context(tc.tile_pool(name="big", bufs=2))
        pp1 = ctx.enter_context(tc.tile_pool(name="ps1", bufs=1, space="PSUM"))
        pp2 = ctx.enter_context(tc.tile_pool(name="ps2", bufs=2, space="PSUM"))
        dpool = ctx.enter_context(tc.tile_pool(name="dram", bufs=2, space="DRAM"))

        def cload(ap_in, shape, tag, dtype=dt.float32):
            t_ = cpool.tile(shape, dtype, tag=tag, name=tag)
            nc.sync.dma_start(t_[:], ap_in)
            return t_

        v128_s = cload(v128, [P, S], tag='c_v128')
        cc_s = cload(cc, [P, 8], tag='c_cc')
        smM_s = cload(segmaskM, [P, T * M], tag='c_smM')
        iop1_s = cload(iop1T, [P, T * M], tag='c_iop1')
        s256_s = cload(seg256T, [P, T * M], tag='c_s256')
        sm62_s = cload(segmask62, [P, T * 62], tag='c_sm62')
        iev_s = cload(iev62T, [P, T * 62], tag='c_iev')
        oneS_s = cload(oneSst, [P, T * S], tag='c_oneS')
        zeroM_s = cload(zeroM, [P, M], tag='c_zeroM')
        lhsT6_s = cload(lhsT6, [6, 128], tag='c_lhsT6', dtype=dt.float32r)
        b1_s = cload(b1col, [128, 1], tag='c_b1col')
        w0p_s = cload(w0pair, [128, 2], tag='c_w0pair', dtype=dt.bfloat16)
        wgc_s = cload(wgcpair, [128, 128], tag='c_wgc', dtype=dt.bfloat16)
        wc2_s = cload(wc2pair, [128, 6], tag='c_wc2', dtype=dt.float32r)
        dlhs_s = cload(dlhs, [4, 64], tag='c_dlhs')
        bc2_s = cload(bc2col, [6, 1], tag='c_bc2col')
        bg_s = cload(bgrep, [P, 3], tag='c_bgrep')
        scl_s = cload(scl_in, [P, 4], tag='c_scl')
        ro_s = cload(rays_o, [P, T, 3], tag='c_rays_o')
        rd_s = cload(rays_d, [P, T, 3], tag='c_rays_d')

        ones_c = cc_s[:, 0:1]
        eps_c = cc_s[:, 1:2]
        e15_c = cc_s[:, 2:3]
        e5_c = cc_s[:, 3:4]
        nhalf_c = cc_s[:, 4:5]
        m24_c = cc_s[:, 5:6]
        nm24_c = cc_s[:, 6:7]
        n8003_c = cc_s[:, 7:8]
        bd2_0c = scl_s[:, 0:1]

        def bc(col, n):
            return col.broadcast_to((P, n))

        # ============ STAGE A: ray setup (batched over T) ============
        n24 = T * 3

        def st(shape, tag, dtype=dt.float32):
            return spool.tile(shape, dtype, tag=tag, name=tag)

        negd = st([P, T, 3], 's_negd')
        nc.vector.tensor_scalar(negd[:], rd_s[:], -1.0, None, Alu.mult)
        absd = st([P, T, 3], 's_absd')
        nc.vector.tensor_tensor(absd[:], rd_s[:], negd[:], Alu.max)
        dmask = st([P, T, 3], 's_dmask', dt.uint8)
        nc.vector.tensor_scalar(dmask[:], absd[:], 1e-9, None, Alu.is_lt)
        dsafe = st([P, T, 3], 's_dsafe')
        nc.vector.select(dsafe[:].rearrange("p t c -> p (t c)"),
                         dmask[:].rearrange("p t c -> p (t c)"),
                         bc(eps_c, n24),
                         rd_s[:].rearrange("p t c -> p (t c)"))
        invd = st([P, T, 3], 's_invd')
        nc.vector.reciprocal(invd[:], dsafe[:])
        a1 = st([P, T, 3], 's_a1')
        nc.vector.scalar_tensor_tensor(a1[:], ro_s[:], 1.0, invd[:],
                                       Alu.add, Alu.mult)
        b1 = st([P, T, 3], 's_b1')
        nc.vector.scalar_tensor_tensor(b1[:], ro_s[:], -1.0, invd[:],
                                       Alu.add, Alu.mult)
        mx = st([P, T, 3], 's_mx')
        nc.vector.tensor_tensor(mx[:], a1[:], b1[:], Alu.max)
        mn = st([P, T, 3], 's_mn')
        nc.vector.tensor_tensor(mn[:], a1[:], b1[:], Alu.min)
        tmin = st([P, T], 's_tmin')
        nc.vector.tensor_reduce(tmin[:], mx[:], AxX, Alu.min)
        tmax = st([P, T], 's_tmax')
        nc.vector.tensor_reduce(tmax[:], mn[:], AxX, Alu.max)
        near = st([P, T], 's_near')
        nc.vector.tensor_scalar(near[:], tmin[:], -1.0, MIN_NEAR,
                                Alu.mult, Alu.max)
        tmaxt = st([P, T], 's_tmaxt')
        nc.vector.tensor_scalar(tmaxt[:], tmax[:], -1.0, None, Alu.mult)
        fmask = st([P, T], 's_fmask', dt.uint8)
        nc.vector.tensor_tensor(fmask[:], tmaxt[:], near[:], Alu.is_lt)
        nearp = st([P, T], 's_nearp')
        nc.vector.tensor_scalar(nearp[:], near[:], 1e-2, None, Alu.add)
        far = st([P, T], 's_far')
        nc.vector.select(far[:], fmask[:], nearp[:], tmaxt[:])
        rng = st([P, T], 's_rng')
        nc.vector.tensor_tensor(rng[:], far[:], near[:], Alu.subtract)
        dzv = st([P, T], 's_dzv')
        nc.vector.tensor_scalar(dzv[:], rng[:], 1.0 / 63.0, None, Alu.mult)
        sdv = st([P, T], 's_sdv')
        nc.vector.tensor_scalar(sdv[:], rng[:], 1.0 / 64.0, None, Alu.mult)
        invdz = st([P, T], 's_invdz')
        nc.vector.reciprocal(invdz[:], dzv[:])
        inv2dz = st([P, T], 's_inv2dz')
        nc.vector.tensor_scalar(inv2dz[:], invdz[:], 2.0, None, Alu.mult)
        mid0 = st([P, T], 's_mid0')
        nc.vector.scalar_tensor_tensor(mid0[:], dzv[:], 0.5, near[:],
                                       Alu.mult, Alu.add)
        aoff = st([P, T], 's_aoff')
        nc.vector.tensor_tensor(aoff[:], near[:], dzv[:], Alu.subtract)

        # coarse z grid, batched: zc3[p,t,s] = near[p,t] + v128[s]*rng[p,t]
        zc3 = st([P, T, S], 's_zc3')
        v3 = v128_s[:].rearrange("p (o s) -> p o s", o=1).broadcast_to((P, T, S))
        rng3 = rng[:].rearrange("p (t o) -> p t o", o=1).broadcast_to((P, T, S))
        near3 = near[:].rearrange("p (t o) -> p t o", o=1).broadcast_to((P, T, S))
        nc.vector.tensor_tensor(zc3[:], v3, rng3, Alu.mult)
        nc.vector.tensor_tensor(zc3[:], zc3[:], near3, Alu.add)

        # persistent cross-phase arrays
        h20T = st([P, T, S], 's_h20T')
        Z3 = st([P, T, M], 's_Z3')
        dl3 = st([P, T, M], 's_dl3')

        def b3(col2, n):
            # [P, MB] -> [P, MB, n] broadcast
            return col2.rearrange("p (t o) -> p t o", o=1) \
                .broadcast_to((P, MB, n))

        # ================= PHASE 1: coarse MLP + sigma =================
        for t in range(T):
            xyzc = wpool.tile([P, 3, S], dt.float32r, tag="xyzc", name="xyzc")
            for c in range(3):
                nc.vector.scalar_tensor_tensor(
                    xyzc[:, c, :], zc3[:, t, :], rd_s[:, t, c:c + 1],
                    bc(ro_s[:, t, c:c + 1], S), Alu.mult, Alu.add)
            scr = dpool.tile([3, P, S], dt.float32r, tag="xyzscr", name="xyzscr")
            nc.sync.dma_start(scr[:].rearrange("c p s -> p c s"), xyzc[:])
            rhs6 = rpool.tile([6, P * S // 2], dt.float32r, tag="rhs6c",
                              name="rhs6c")
            scrf = scr[:].rearrange("c p s -> c (p s)")
            half = P * S // 2
            nc.sync.dma_start(rhs6[0:3, :], scrf[:, 0:half])
            nc.sync.dma_start(rhs6[3:6, :], scrf[:, half:2 * half])

            sgs = dpool.tile([2, 64, S], dt.float32, tag="sigscr", name="sigscr")
            sgsf = sgs[:].rearrange("h p s -> (h p s)")

            pS = None
            for hf in range(4):          # 4 half-groups of 1024 cols
                pA = pp1.tile([128, 1024], dt.float32, tag="pA", name="pA")
                for c2 in range(2):
                    mmr(pA[:, 512 * c2:512 * (c2 + 1)], lhsT6_s[:],
                        rhs6[:, 1024 * hf + 512 * c2:1024 * hf + 512 * (c2 + 1)],
                        start=True, stop=True)
                rh1 = bpool.tile([128, 1024], dt.bfloat16, tag="rh1", name="rh1")
                if hf % 2 == 0:
                    nc.vector.tensor_scalar(rh1[:], pA[:], b1_s[:], 0.0,
                                            Alu.add, Alu.max)
                else:
                    nc.scalar.activation(rh1[:], pA[:], Act.Relu, bias=b1_s[:])
                if hf % 2 == 0:
                    pS = pp1.tile([128, 512], dt.float32, tag="pS", name="pS")
                    if SIM_SAFE:
                        nc.vector.memset(pS[:], 0.0)
                for c2 in range(2):
                    cc_g = 2 * (hf % 2) + c2
                    pos = 32 * cc_g
                    nc.tensor.matmul(pS[pos:pos + 2, :], w0p_s[:],
                                     rh1[:, 512 * c2:512 * (c2 + 1)],
                                     start=True, stop=True,
                                     tile_position=(0, pos))
                if hf % 2 == 1:
                    ps_i = hf // 2
                    sg_sb = bpool.tile([128, 512], dt.float32, tag="sgsb",
                                       name="sgsb")
                    if ps_i == 0:
                        nc.vector.tensor_copy(sg_sb[:], pS[:])
                    else:
                        nc.scalar.copy(sg_sb[:], pS[:])
                    # rows (32a+h) -> dram (h, p=32*ps+8a+q, s)
                    for a_ in range(4):
                        dst = sgs[:, 32 * ps_i + 8 * a_:
                                  32 * ps_i + 8 * (a_ + 1), :] \
                            .rearrange("h p s -> h (p s)")
                        nc.sync.dma_start(dst, sg_sb[32 * a_:32 * a_ + 2, :])
            nc.sync.dma_start(h20T[:, t, :],
                              sgs[:].rearrange("h p s -> (h p) s"))

        # ================= machinery (batched per MB tiles) =================
        def mt(shape, tag, dtype=dt.float32):
            return mpool.tile(shape, dtype, tag=tag, name=tag)

        for mb in range(NG):
            t0 = mb * MB
            colM = slice(t0 * M, (t0 + MB) * M)
            colS = slice(t0 * S, (t0 + MB) * S)
            col62 = slice(t0 * 62, (t0 + MB) * 62)
            h20v = h20T[:, t0:t0 + MB, :]
            dz_sl = dzv[:, t0:t0 + MB]
            sd_sl = sdv[:, t0:t0 + MB]
            near_sl = near[:, t0:t0 + MB]
            i2dz_sl = inv2dz[:, t0:t0 + MB]
            mid0_sl = mid0[:, t0:t0 + MB]
            aoff_sl = aoff[:, t0:t0 + MB]

            TM2 = MB * M

            # --- coarse composite weights ---
            sig3 = mt([P, MB, S], 'm_sig3')
            nc.scalar.activation(sig3[:].rearrange("p t s -> p (t s)"),
                                 h20v.rearrange("p t s -> p (t s)"),
                                 Act.Exp, bias=bd2_0c)
            dsgc = mt([P, MB, S], 'm_dsgc')
            nc.vector.tensor_tensor(dsgc[:], sig3[:], b3(dz_sl, S), Alu.mult)
            nc.vector.tensor_tensor(dsgc[:, :, S - 1:S], sig3[:, :, S - 1:S],
                                    b3(sd_sl, 1), Alu.mult)
            emc = mt([P, MB, S], 'm_emc')
            nc.scalar.activation(emc[:].rearrange("p t s -> p (t s)"),
                                 dsgc[:].rearrange("p t s -> p (t s)"),
                                 Act.Exp, scale=-1.0)
            d0c = mt([P, MB, S], 'm_d0c')
            nc.vector.memset(d0c[:, :, 0:1], 0.0)
            nc.scalar.activation(d0c[:, :, 1:S], emc[:, :, 0:S - 1],
                                 Act.Identity, bias=e15_c)
            Tc = mt([P, MB, S], 'm_Tc')
            nc.vector.tensor_tensor_scan(
                Tc[:].rearrange("p t s -> p (t s)"),
                d0c[:].rearrange("p t s -> p (t s)"),
                oneS_s[:, colS], 0.0, Alu.mult, Alu.add)
            alpha = mt([P, MB, S], 'm_alpha')
            nc.scalar.activation(alpha[:].rearrange("p t s -> p (t s)"),
                                 emc[:].rearrange("p t s -> p (t s)"),
                                 Act.Identity, scale=-1.0, bias=ones_c)
            wts = mt([P, MB, S], 'm_wts')
            nc.vector.tensor_tensor(wts[:], alpha[:], Tc[:], Alu.mult)

            # --- pdf/cdf over weights[:,1:63] ---
            wp = mt([P, MB, 62], 'm_wp')
            nc.scalar.activation(wp[:], wts[:, :, 1:63], Act.Identity,
                                 bias=e5_c)
            ssum = mt([P, MB], 'm_ssum')
            nc.vector.tensor_reduce(ssum[:], wp[:], AxX, Alu.add)
            pinv = mt([P, MB], 'm_pinv')
            nc.vector.reciprocal(pinv[:], ssum[:])
            pdf = mt([P, MB, 62], 'm_pdf')
            nc.vector.tensor_tensor(pdf[:], wp[:], b3(pinv[:], 62), Alu.mult)
            cdf = mt([P, MB, 62], 'm_cdf')
            nc.vector.tensor_tensor_scan(
                cdf[:].rearrange("p t s -> p (t s)"), sm62_s[:, col62],
                pdf[:].rearrange("p t s -> p (t s)"), 0.0, Alu.mult, Alu.add)

            # --- scatter cdf onto per-segment 128-slot timelines ---
            r2 = mt([P, MB, 62], 'm_r2')
            r2f = r2[:].rearrange("p t s -> p (t s)")
            cdff = cdf[:].rearrange("p t s -> p (t s)")
            nc.scalar.activation(r2f, cdff, Act.Identity, scale=128.0,
                                 bias=m24_c)
            nc.scalar.activation(r2f, r2f, Act.Identity, bias=nm24_c)
            idx2f = mt([P, MB, 124], 'm_idx2f')
            i4 = idx2f[:].rearrange("p t (a b) -> p t a b", b=2)
            ev = i4[:, :, :, 0:1].rearrange("p t a b -> p t (a b)")
            od = i4[:, :, :, 1:2].rearrange("p t a b -> p t (a b)")
            nc.vector.tensor_tensor(
                ev, r2[:], iev_s[:, col62].rearrange("p (t s) -> p t s", t=MB),
                Alu.add)
            nc.scalar.activation(od, ev, Act.Identity, bias=ones_c)
            idx2i = mt([P, MB * 124], 'm_idx2i', dt.int16)
            nc.scalar.copy(idx2i[:], idx2f[:].rearrange("p t s -> p (t s)"))
            tlc2 = mt([P, MB * 256], 'm_tlc2', dt.int16)
            nc.gpsimd.local_scatter(tlc2[:], cdff.bitcast(dt.int16),
                                    idx2i[:], channels=P,
                                    num_elems=MB * 256, num_idxs=MB * 124)
            tlc = tlc2[:].bitcast(dt.float32)
            tlc3 = tlc.rearrange("p (t s) -> p t s", t=MB)

            # --- fills and counts on the timeline ---
            notC = mt([P, MB, M], 'm_notC')
            notCf = notC[:].rearrange("p t s -> p (t s)")
            nc.vector.tensor_scalar(notCf, tlc, 0.0, None, Alu.is_equal)
            notCp = mt([P, MB * M], 'm_notCp')
            nc.gpsimd.tensor_tensor(notCp[:], notCf, smM_s[:, colM], Alu.mult)
            kp1 = mt([P, MB, M], 'm_kp1')
            kp1f = kp1[:].rearrange("p t s -> p (t s)")
            nc.vector.tensor_tensor_scan(kp1f, smM_s[:, colM], notCf,
                                         0.0, Alu.mult, Alu.add)
            uu = mt([P, MB * M], 'm_uu')
            nc.scalar.activation(uu[:], kp1f, Act.Identity,
                                 scale=1.0 / 64.0, bias=nhalf_c)
            cntC = mt([P, MB * M], 'm_cntC')
            nc.vector.tensor_tensor(cntC[:], iop1_s[:, colM], kp1f,
                                    Alu.subtract)
            ffwd = mt([P, MB * M], 'm_ffwd')
            nc.vector.tensor_tensor_scan(ffwd[:], notCp[:], tlc, 0.0,
                                         Alu.mult, Alu.add)
            rnotC = mt([P, MB, M], 'm_rnotC')
            nc.scalar.copy(rnotC[:], notC[:, :, ::-1])
            rnotCp = mt([P, MB * M], 'm_rnotCp')
            nc.gpsimd.tensor_tensor(rnotCp[:],
                                    rnotC[:].rearrange("p t s -> p (t s)"),
                                    smM_s[:, colM], Alu.mult)
            rtlc = mt([P, MB, M], 'm_rtlc')
            nc.scalar.copy(rtlc[:], tlc3[:, :, ::-1])
            rbwd = mt([P, MB, M], 'm_rbwd')
            nc.vector.tensor_tensor_scan(
                rbwd[:].rearrange("p t s -> p (t s)"), rnotCp[:],
                rtlc[:].rearrange("p t s -> p (t s)"), 0.0, Alu.mult, Alu.add)

            # --- inverse-CDF lerp at u slots ---
            den = mt([P, MB, M], 'm_den')
            nc.vector.tensor_tensor(den[:], rbwd[:, :, ::-1],
                                    ffwd[:].rearrange("p (t s) -> p t s", t=MB),
                                    Alu.subtract)
            denf = den[:].rearrange("p t s -> p (t s)")
            mkd = mt([P, MB * M], 'm_mkd', dt.uint8)
            nc.vector.tensor_scalar(mkd[:], denf, 1e-5, None, Alu.is_lt)
            nc.vector.select(denf, mkd[:], bc(ones_c, TM2), denf)
            rden = mt([P, MB * M], 'm_rden')
            nc.vector.reciprocal(rden[:], denf)
            ttv = mt([P, MB * M], 'm_ttv')
            nc.vector.tensor_tensor(ttv[:], uu[:], ffwd[:], Alu.subtract)
            nc.vector.tensor_tensor(ttv[:], ttv[:], rden[:], Alu.mult)
            bg0 = mt([P, MB, M], 'm_bg0')
            nc.gpsimd.tensor_tensor(bg0[:],
                                    cntC[:].rearrange("p (t s) -> p t s", t=MB),
                                    b3(dz_sl, M), Alu.mult)
            nc.gpsimd.tensor_tensor(bg0[:], bg0[:], b3(mid0_sl, M), Alu.add)
            nz = mt([P, MB, M], 'm_nz')
            nc.vector.tensor_tensor(nz[:],
                                    ttv[:].rearrange("p (t s) -> p t s", t=MB),
                                    b3(dz_sl, M), Alu.mult)
            nc.vector.tensor_tensor(nz[:], nz[:], bg0[:], Alu.add)
            nzf = nz[:].rearrange("p t s -> p (t s)")

            # --- merge ranks into final (coarse ∪ fine) timeline ---
            q2 = mt([P, MB, M], 'm_q2')
            nc.vector.tensor_tensor(q2[:], nz[:], b3(near_sl, M), Alu.subtract)
            nc.vector.tensor_tensor(q2[:], q2[:], b3(i2dz_sl, M), Alu.mult)
            q2f = q2[:].rearrange("p t s -> p (t s)")
            nc.vector.tensor_scalar(q2f, q2f, 1.0, M24, Alu.add, Alu.add)
            nc.vector.tensor_scalar(q2f, q2f, M24, 0.0, Alu.subtract, Alu.max)
            nc.vector.tensor_scalar(q2f, q2f, 126.0, 8001.0, Alu.min, Alu.add)
            tk2 = mt([P, MB * M], 'm_tk2')
            nc.scalar.activation(tk2[:], kp1f, Act.Identity, scale=2.0,
                                 bias=n8003_c)
            mk2 = mt([P, MB * M], 'm_mk2')
            nc.vector.tensor_scalar(mk2[:], kp1f, 64.5, None, Alu.is_gt)
            minv = mt([P, MB * M], 'm_minv')
            nc.vector.scalar_tensor_tensor(minv[:], mk2[:], 1.0, notCf,
                                           Alu.add, Alu.subtract)
            m2 = mt([P, MB * M], 'm_m2')
            nc.vector.scalar_tensor_tensor(m2[:], minv[:], -4000.0, q2f,
                                           Alu.mult, Alu.add)
            ms = mt([P, MB * M], 'm_ms')
            nc.vector.tensor_tensor_scan(ms[:], smM_s[:, colM], m2[:],
                                         0.0, Alu.mult, Alu.max)
            rkv = mt([P, MB * M], 'm_rkv')
            nc.vector.tensor_tensor(rkv[:], tk2[:], ms[:], Alu.add)
            nc.vector.tensor_scalar(rkv[:], rkv[:], 254.0, None, Alu.min)
            nc.vector.scalar_tensor_tensor(rkv[:], minv[:], -4000.0, rkv[:],
                                           Alu.mult, Alu.add)
            fidx2f = mt([P, MB, 256], 'm_fidx2f')
            f4 = fidx2f[:].rearrange("p t (a b) -> p t a b", b=2)
            fev = f4[:, :, :, 0:1].rearrange("p t a b -> p t (a b)")
            fod = f4[:, :, :, 1:2].rearrange("p t a b -> p t (a b)")
            nc.vector.tensor_tensor(
                fev, rkv[:].rearrange("p (t s) -> p t s", t=MB),
                s256_s[:, colM].rearrange("p (t s) -> p t s", t=MB), Alu.add)
            nc.scalar.activation(fod, fev, Act.Identity, bias=ones_c)
            fidx2i = mt([P, MB * 256], 'm_fidx2i', dt.int16)
            nc.scalar.copy(fidx2i[:], fidx2f[:].rearrange("p t s -> p (t s)"))
            zf2 = mt([P, MB * 256], 'm_zf2', dt.int16)
            nc.gpsimd.local_scatter(zf2[:], nzf.bitcast(dt.int16),
                                    fidx2i[:], channels=P,
                                    num_elems=MB * 256, num_idxs=MB * 256)
            zsc = zf2[:].bitcast(dt.float32)

            # --- fill coarse slots with uniform grid ---
            isCC = mt([P, MB, M], 'm_isCC')
            isCCf = isCC[:].rearrange("p t s -> p (t s)")
            nc.vector.tensor_scalar(isCCf, zsc, 0.0, None, Alu.is_equal)
            cum2 = mt([P, MB, M], 'm_cum2')
            nc.vector.tensor_tensor_scan(
                cum2[:].rearrange("p t s -> p (t s)"), smM_s[:, colM],
                isCCf, 0.0, Alu.mult, Alu.add)
            zcf = mt([P, MB, M], 'm_zcf')
            nc.gpsimd.tensor_tensor(zcf[:], cum2[:], b3(dz_sl, M), Alu.mult)
            nc.gpsimd.tensor_tensor(zcf[:], zcf[:], b3(aoff_sl, M), Alu.add)
            Zv = Z3[:, t0:t0 + MB, :]
            nc.vector.tensor_tensor(Zv, isCC[:], zcf[:], Alu.mult)
            nc.vector.tensor_tensor(
                Zv, Zv, zsc.rearrange("p (t s) -> p t s", t=MB), Alu.add)
            dv = dl3[:, t0:t0 + MB, :]
            nc.vector.tensor_tensor(dv[:, :, 0:M - 1], Zv[:, :, 1:M],
                                    Zv[:, :, 0:M - 1], Alu.subtract)
            nc.scalar.copy(dv[:, :, M - 1:M],
                           sd_sl.rearrange("p (t o) -> p t o", o=1))

        # ================= PHASE 2: fine MLP + composite =================
        fh = P * M // 2
        for t in range(T):
            xyzm = wpool.tile([P, 3, M], dt.float32r, tag="xyzm", name="xyzm")
            for c in range(3):
                nc.vector.scalar_tensor_tensor(
                    xyzm[:, c, :], Z3[:, t, :], rd_s[:, t, c:c + 1],
                    bc(ro_s[:, t, c:c + 1], M), Alu.mult, Alu.add)
            scr2 = dpool.tile([3, P, M], dt.float32r, tag="xyzscr2",
                              name="xyzscr2")
            nc.sync.dma_start(scr2[:].rearrange("c p s -> p c s"), xyzm[:])
            dT_sb = wpool.tile([4, P], dt.float32, tag="dTsb", name="dTsb")
            nc.sync.dma_start(dT_sb[:], dT_in[t])
            rhs6f = rpool.tile([6, fh], dt.float32r, tag="rhs6f", name="rhs6f")
            scr2f = scr2[:].rearrange("c p s -> c (p s)")
            nc.sync.dma_start(rhs6f[0:3, :], scr2f[:, 0:fh])
            nc.sync.dma_start(rhs6f[3:6, :], scr2f[:, fh:2 * fh])

            # dterm pairs [128, 64] (v1 form)
            pD = pp1.tile([128, 512], dt.float32, tag="pS", name="pS")
            if SIM_SAFE:
                nc.vector.memset(pD[:], 0.0)
            nc.tensor.matmul(pD[0:64, 0:64], dlhs_s[:], dT_sb[:, 0:64],
                             start=True, stop=True)
            nc.tensor.matmul(pD[64:128, 0:64], dlhs_s[:], dT_sb[:, 64:128],
                             start=True, stop=True, tile_position=(0, 64))
            dtm = bpool.tile([128, 64], dt.float32, tag="dtm", name="dtm")
            nc.scalar.copy(dtm[:], pD[:, 0:64])

            sgs2 = dpool.tile([2, 64, M], dt.float32, tag="sigscr2",
                              name="sigscr2")
            sgs2f = sgs2[:].rearrange("h p s -> (h p s)")
            rgbs = dpool.tile([2, 3, 64, M], dt.float32, tag="rgbscr",
                              name="rgbscr")
            rgbsw = rgbs[:].rearrange("h c p s -> (h c) (p s)")

            for g in range(4):
                rgb_sb = bpool.tile([6, 2048], dt.float32, tag="rgbsb",
                                    name="rgbsb")
                pS = pp1.tile([128, 512], dt.float32, tag="pS", name="pS")
                if SIM_SAFE:
                    nc.vector.memset(pS[:], 0.0)
                for hf in range(2):
                    base = g * 2048 + hf * 1024
                    pA = pp1.tile([128, 1024], dt.float32, tag="pA", name="pA")
                    for c2 in range(2):
                        mmr(pA[:, 512 * c2:512 * (c2 + 1)], lhsT6_s[:],
                            rhs6f[:, base + 512 * c2:base + 512 * (c2 + 1)],
                            start=True, stop=True)
                    rh1f = bpool.tile([128, 1024], dt.bfloat16, tag="rh1",
                                      name="rh1")
                    if hf == 0:
                        nc.vector.tensor_scalar(rh1f[:], pA[:], b1_s[:], 0.0,
                                                Alu.add, Alu.max)
                    else:
                        nc.scalar.activation(rh1f[:], pA[:], Act.Relu,
                                             bias=b1_s[:])
                    for c2 in range(2):
                        cc4 = 2 * hf + c2
                        pos = 32 * cc4
                        nc.tensor.matmul(pS[pos:pos + 2, :], w0p_s[:],
                                         rh1f[:, 512 * c2:512 * (c2 + 1)],
                                         start=True, stop=True,
                                         tile_position=(0, pos))
                    pG = pp2.tile([128, 1024], dt.float32, tag="pG", name="pG")
                    for c2 in range(2):
                        nc.tensor.matmul(pG[:, 512 * c2:512 * (c2 + 1)],
                                         wgc_s[:],
                                         rh1f[:, 512 * c2:512 * (c2 + 1)],
                                         start=True, stop=True)
                    ch1 = bpool.tile([128, 1024], dt.float32r, tag="ch1",
                                     name="ch1")
                    # fused relu(pG + dterm): dterm is constant per
                    # 128-col ray-pair block -> per-block bias column
                    for a_ in range(8):
                        blk = slice(128 * a_, 128 * (a_ + 1))
                        dcol = dtm[:, g * 16 + hf * 8 + a_:
                                   g * 16 + hf * 8 + a_ + 1]
                        if a_ % 2 == 0:
                            nc.scalar.activation(ch1[:, blk], pG[:, blk],
                                                 Act.Relu, bias=dcol)
                        else:
                            nc.vector.tensor_scalar(ch1[:, blk], pG[:, blk],
                                                    dcol, 0.0, Alu.add,
                                                    Alu.max)
                    for c2 in range(2):
                        pC = pp1.tile([6, 512], dt.float32, tag="pC", name="pC")
                        mmr(pC[:], wc2_s[:], ch1[:, 512 * c2:512 * (c2 + 1)],
                            start=True, stop=True)
                        osl = rgb_sb[:, 1024 * hf + 512 * c2:
                                     1024 * hf + 512 * (c2 + 1)]
                        if (2 * hf + c2) % 2 == 0:
                            nc.scalar.activation(osl, pC[:], Act.Identity,
                                                 bias=bc2_s[:])
                        else:
                            nc.vector.tensor_scalar(osl, pC[:], bc2_s[:],
                                                    None, Alu.add)
                # sigma eviction (packed) + DMA
                sg_sb = bpool.tile([128, 512], dt.float32, tag="sgsb",
                                   name="sgsb")
                if g % 2 == 0:
                    nc.vector.tensor_copy(sg_sb[:], pS[:])
                else:
                    nc.scalar.copy(sg_sb[:], pS[:])
                for a_ in range(4):
                    dst = sgs2[:, 16 * g + 4 * a_:16 * g + 4 * (a_ + 1), :] \
                        .rearrange("h p s -> h (p s)")
                    nc.sync.dma_start(dst, sg_sb[32 * a_:32 * a_ + 2, :])
                nc.sync.dma_start(rgbsw[:, g * 2048:(g + 1) * 2048], rgb_sb[:])

            # ---- composite in rays layout ----
            def wt(shape, tag, dtype=dt.float32):
                return wpool.tile(shape, dtype, tag=tag, name=tag)

            h20m = wt([P, M], "h20m")
            nc.sync.dma_start(h20m[:], sgs2[:].rearrange("h p s -> (h p) s"))
            sigm = wt([P, M], "sigm")
            nc.scalar.activation(sigm[:], h20m[:], Act.Exp, bias=bd2_0c)
            dsg2 = wt([P, M], "dsg2")
            nc.vector.tensor_tensor(dsg2[:], dl3[:, t, :], sigm[:], Alu.mult)
            em2 = wt([P, M], "em2")
            nc.scalar.activation(em2[:], dsg2[:], Act.Exp, scale=-1.0)
            sb2 = wt([P, M], "sb2")
            nc.vector.memset(sb2[:, 0:1], 1.0)
            nc.scalar.activation(sb2[:, 1:M], em2[:, 0:M - 1],
                                 Act.Identity, bias=e15_c)
            Tm = wt([P, M], "Tm")
            nc.vector.tensor_tensor_scan(Tm[:], sb2[:], zeroM_s[:], 1.0,
                                         Alu.mult, Alu.add)
            alpm = wt([P, M], "alpm")
            nc.scalar.activation(alpm[:], em2[:], Act.Identity,
                                 scale=-1.0, bias=ones_c)
            wm = wt([P, M], "wm")
            nc.vector.tensor_tensor(wm[:], alpm[:], Tm[:], Alu.mult)
            wsum = wt([P, 1], "wsum")
            nc.vector.tensor_reduce(wsum[:], wm[:], AxX, Alu.add)
            wmm = wt([P, M], "wmm")
            nc.vector.scalar_tensor_tensor(wmm[:], wm[:], 1e-4, wm[:],
                                           Alu.is_gt, Alu.mult)

            rgbp = wt([P, 3, M], "rgbp")
            for h_ in range(2):
                nc.sync.dma_start(rgbp[64 * h_:64 * (h_ + 1), :, :],
                                  rgbs[h_].rearrange("c p s -> p c s"))
            erg = wt([P, 3, M], "erg")
            nc.scalar.activation(erg[:].rearrange("p c s -> p (c s)"),
                                 rgbp[:].rearrange("p c s -> p (c s)"),
                                 Act.Exp, scale=-1.0)
            nc.scalar.activation(erg[:].rearrange("p c s -> p (c s)"),
                                 erg[:].rearrange("p c s -> p (c s)"),
                                 Act.Identity, bias=ones_c)
            rgbv = wt([P, 3, M], "rgbv")
            nc.vector.reciprocal(rgbv[:], erg[:])
            nc.vector.tensor_tensor(
                rgbv[:], rgbv[:],
                wmm[:].rearrange("p (o s) -> p o s", o=1)
                .broadcast_to((P, 3, M)), Alu.mult)
            img = wt([P, 3], "img")
            nc.vector.tensor_reduce(img[:], rgbv[:], AxX, Alu.add)
            bgw = wt([P, 1], "bgw")
            nc.vector.tensor_scalar(bgw[:], wsum[:], -1.0, 1.0, Alu.mult,
                                    Alu.add)
            nc.vector.scalar_tensor_tensor(img[:], bg_s[:], bgw[:], img[:],
                                           Alu.mult, Alu.add)
            nc.sync.dma_start(img_out[:, t, :], img[:])

    nc.compile()
    return nc


def _host_constants(inputs):
    Wd1 = np.asarray(inputs["Wd1"], np.float32)
    bd1 = np.asarray(inputs["bd1"], np.float32)
    Wd2 = np.asarray(inputs["Wd2"], np.float32)
    bd2 = np.asarray(inputs["bd2"], np.float32)
    Wc1 = np.asarray(inputs["Wc1"], np.float32)
    bc1 = np.asarray(inputs["bc1"], np.float32)
    Wc2 = np.asarray(inputs["Wc2"], np.float32)
    bc2 = np.asarray(inputs["bc2"], np.float32)
    tval = float(np.asarray(inputs["time"]).reshape(()))

    W1 = Wd1[:3]
    b1p = bd1 + tval * Wd1[3]
    w0 = Wd2[:, 0:1]
    Wgc = (Wd2[:, 1:].astype(np.float64) @ Wc1[3:].astype(np.float64)) \
        .astype(np.float32)
    bgc = (bd2[1:].astype(np.float64) @ Wc1[3:].astype(np.float64)) \
        .astype(np.float32)
    bd2_0 = float(bd2[0])

    lhsT6 = np.zeros((6, 128), np.float32)
    lhsT6[0:3, 0:64] = W1
    lhsT6[3:6, 64:128] = W1
    b1col = np.concatenate([b1p, b1p]).reshape(128, 1).astype(np.float32)

    w0pair = np.zeros((128, 2), np.float32)
    w0pair[0:64, 0:1] = w0
    w0pair[64:128, 1:2] = w0

    wgcpair = np.zeros((128, 128), np.float32)
    wgcpair[0:64, 0:64] = Wgc
    wgcpair[64:128, 64:128] = Wgc

    wc2pair = np.zeros((128, 6), np.float32)
    wc2pair[0:64, 0:3] = Wc2
    wc2pair[64:128, 3:6] = Wc2

    dlhs = np.zeros((4, 64), np.float32)
    dlhs[0:3] = Wc1[:3]
    dlhs[3] = bc1 + bgc

    bc2col = np.concatenate([bc2, bc2]).reshape(6, 1).astype(np.float32)


    v = np.linspace(0.0, 1.0, S, dtype=np.float32)

    jM = np.arange(T * M)
    j62 = np.arange(T * 62)
    jS = np.arange(T * S)
    segmaskM = (jM % M != 0).astype(np.float32)
    iop1T = (jM % M + 1).astype(np.float32)
    seg256T = (256 * ((jM // M) % MB)).astype(np.float32)
    segmask62 = (j62 % 62 != 0).astype(np.float32)
    iev62T = (2 * (j62 % 62) + 256 * ((j62 // 62) % MB)).astype(np.float32)
    oneSst = (jS % S == 0).astype(np.float32)

    def rep(row):
        return np.broadcast_to(row, (P,) + row.shape).copy()

    return {
        "v128": rep(v),
        "cc": rep(np.array([1.0, 1e-9, 1e-15, 1e-5, -1.0 / 128.0,
                    16777216.0, -16777216.0, -8003.0],
                   np.float32)),
        "segmaskM": rep(segmaskM),
        "iop1T": rep(iop1T),
        "seg256T": rep(seg256T),
        "segmask62": rep(segmask62),
        "iev62T": rep(iev62T),
        "oneSst": rep(oneSst),
        "zeroM": np.zeros((P, M), np.float32),
        "lhsT6": lhsT6, "b1col": b1col,
        "w0pair": w0pair.astype(ml_dtypes.bfloat16),
        "wgcpair": wgcpair.astype(ml_dtypes.bfloat16),
        "wc2pair": wc2pair, "dlhs": dlhs,
        "bc2col": bc2col,
        "bgrep": np.broadcast_to(
            np.asarray(inputs["background_color"], np.float32), (P, 3)).copy(),
        "scl": np.broadcast_to(
            np.array([bd2_0, 0, 0, 0], np.float32), (P, 4)).copy(),
    }


def kernel(**inputs):
    global _BUILT
    assert int(inputs["num_steps"]) == S
    assert int(inputs["upsample_steps"]) == U

    if _BUILT is None:
        _BUILT = _build()
    nc = _BUILT

    consts = _host_constants(inputs)
    ro = np.asarray(inputs["rays_o"], np.float32).reshape(NRAYS, 3)
    rd = np.asarray(inputs["rays_d"], np.float32).reshape(NRAYS, 3)

    in_maps = []
    for c in range(NCORES):
        sl_o = ro[c * R:(c + 1) * R].reshape(T, P, 3)
        sl_d = rd[c * R:(c + 1) * R].reshape(T, P, 3)
        dT = np.ones((T, 4, P), np.float32)
        dT[:, 0:3, :] = sl_d.transpose(0, 2, 1)
        m = {
            "rays_o_k": np.ascontiguousarray(sl_o.transpose(1, 0, 2)),
            "rays_d_k": np.ascontiguousarray(sl_d.transpose(1, 0, 2)),
            "dT_k": dT,
        }
        m.update(consts)
        in_maps.append(m)

    res = run_bass_kernel_spmd(nc, in_maps, core_ids=list(range(NCORES)))
    global LAST_RESULT
    LAST_RESULT = res
    outs = []
    for c in range(NCORES):
        img = res.results[c]["img_k"]
        outs.append(img.transpose(1, 0, 2).reshape(R, 3))
    return np.concatenate(outs, 0).reshape(1, NRAYS, 3)


# revision 32
# speedup vs baseline: 1.0140x; 1.0025x over previous
"""DNeRF renderer on 8 Trainium2 cores (Bass/Tile) — v2.

Data-parallel over rays (1024 rays/core, 8 ray-tiles of 128 rays).

v2 structure (vs v1):
- All big matmuls in float32r (1 cyc/row on the PE, full-rate fp32).
- Two-phase schedule: phase 1 = coarse MLP + sigma for all 8 ray-tiles,
  then the per-ray sort/searchsorted machinery BATCHED over pairs of
  ray-tiles (segmented scans via (mult,add)-reset tricks), then phase 2 =
  fine MLP + compositing per ray-tile.  PE streams matmuls back-to-back
  while DVE/Act/Pool run the machinery of earlier tiles.
- Sigma eviction packed into [128,512] psum via tile_position 32-blocks
  (free-size-bound engine cost drops 4x).
- Layer-1 bias folded into the relu eviction (per-partition bias column),
  dropping the ones-row from the matmul rhs.
- bc2 bias folded into the rgb eviction copies.
- dterm broadcast-add runs on the (otherwise idle) Pool engine.
"""

import numpy as np
import ml_dtypes
from contextlib import ExitStack

import concourse.bass as bass
import concourse.bacc as bacc
import concourse.mybir as mybir
import concourse.tile as tile
from concourse.bass_utils import run_bass_kernel_spmd
from concourse import library_config

dt = mybir.dt
Alu = mybir.AluOpType
Act = mybir.ActivationFunctionType
AxX = mybir.AxisListType.X

NCORES = 8
NRAYS = 8192
R = NRAYS // NCORES      # rays per core
P = 128                  # rays per tile (partitions)
T = R // P               # ray-tiles per core
S = 64                   # num_steps
U = 64                   # upsample_steps
M = S + U                # merged samples
MB = 2                   # ray-tiles per machinery batch
NG = T // MB
MIN_NEAR = 0.05
M24 = 16777216.0         # 2^24

_BUILT = None
SIM_SAFE = False


def _build():
    nc = bacc.Bacc("TRN2", target_bir_lowering=False, debug=False,
                   num_devices=NCORES)

    def din(name, shape, dtype=dt.float32):
        return nc.dram_tensor(name, shape, dtype, kind="ExternalInput").ap()

    rays_o = din("rays_o_k", [P, T, 3])
    rays_d = din("rays_d_k", [P, T, 3])
    dT_in = din("dT_k", [T, 4, P])
    v128 = din("v128", [P, S])
    cc = din("cc", [P, 8])
    segmaskM = din("segmaskM", [P, T * M])
    iop1T = din("iop1T", [P, T * M])
    seg256T = din("seg256T", [P, T * M])
    segmask62 = din("segmask62", [P, T * 62])
    iev62T = din("iev62T", [P, T * 62])
    oneSst = din("oneSst", [P, T * S])
    zeroM = din("zeroM", [P, M])
    lhsT6 = din("lhsT6", [6, 128], dt.float32r)
    b1col = din("b1col", [128, 1])
    w0pair = din("w0pair", [128, 2], dt.bfloat16)
    wgcpair = din("wgcpair", [128, 128], dt.bfloat16)
    wc2pair = din("wc2pair", [128, 6], dt.float32r)
    dlhs = din("dlhs", [4, 64])
    bc2col = din("bc2col", [6, 1])
    bgrep = din("bgrep", [P, 3])
    scl_in = din("scl", [P, 4])

    img_out = nc.dram_tensor("img_k", [P, T, 3], dt.float32,
                             kind="ExternalOutput").ap()


    def dep0(ap_):
        # partition-strided APs confuse subtile dep tracking; anchor the
        # tracked range at offset 0 so read/write overlap is detected.
        return bass.AP(tensor=ap_.tensor, offset=ap_.offset, ap=ap_.ap,
                       dep_tracking_offset=0)

    def mmr(out, lhsT, rhs, **kw):
        nc.tensor.matmul(out, lhsT.bitcast(dt.float32r),
                         rhs.bitcast(dt.float32r), **kw)

    with tile.TileContext(nc) as tc, ExitStack() as ctx:

        cpool = ctx.enter_context(tc.tile_pool(name="consts", bufs=1))
        spool = ctx.enter_context(tc.tile_pool(name="setup", bufs=1))
        mpool = ctx.enter_context(tc.tile_pool(name="mach", bufs=1))
        wpool = ctx.enter_context(tc.tile_pool(name="work", bufs=2))
        rpool = ctx.enter_context(tc.tile_pool(name="rhs", bufs=1))
        bpool = ctx.enter_context(tc.tile_pool(name="big", bufs=2))
        pp1 = ctx.enter_context(tc.tile_pool(name="ps1", bufs=1, space="PSUM"))
        pp2 = ctx.enter_context(tc.tile_pool(name="ps2", bufs=2, space="PSUM"))
        dpool = ctx.enter_context(tc.tile_pool(name="dram", bufs=2, space="DRAM"))

        def cload(ap_in, shape, tag, dtype=dt.float32):
            t_ = cpool.tile(shape, dtype, tag=tag, name=tag)
            nc.sync.dma_start(t_[:], ap_in)
            return t_

        v128_s = cload(v128, [P, S], tag='c_v128')
        cc_s = cload(cc, [P, 8], tag='c_cc')
        smM_s = cload(segmaskM, [P, T * M], tag='c_smM')
        iop1_s = cload(iop1T, [P, T * M], tag='c_iop1')
        s256_s = cload(seg256T, [P, T * M], tag='c_s256')
        sm62_s = cload(segmask62, [P, T * 62], tag='c_sm62')
        iev_s = cload(iev62T, [P, T * 62], tag='c_iev')
        oneS_s = cload(oneSst, [P, T * S], tag='c_oneS')
        zeroM_s = cload(zeroM, [P, M], tag='c_zeroM')
        lhsT6_s = cload(lhsT6, [6, 128], tag='c_lhsT6', dtype=dt.float32r)
        b1_s = cload(b1col, [128, 1], tag='c_b1col')
        w0p_s = cload(w0pair, [128, 2], tag='c_w0pair', dtype=dt.bfloat16)
        wgc_s = cload(wgcpair, [128, 128], tag='c_wgc', dtype=dt.bfloat16)
        wc2_s = cload(wc2pair, [128, 6], tag='c_wc2', dtype=dt.float32r)
        dlhs_s = cload(dlhs, [4, 64], tag='c_dlhs')
        bc2_s = cload(bc2col, [6, 1], tag='c_bc2col')
        bg_s = cload(bgrep, [P, 3], tag='c_bgrep')
        scl_s = cload(scl_in, [P, 4], tag='c_scl')
        ro_s = cload(rays_o, [P, T, 3], tag='c_rays_o')
        rd_s = cload(rays_d, [P, T, 3], tag='c_rays_d')

        ones_c = cc_s[:, 0:1]
        eps_c = cc_s[:, 1:2]
        e15_c = cc_s[:, 2:3]
        e5_c = cc_s[:, 3:4]
        nhalf_c = cc_s[:, 4:5]
        m24_c = cc_s[:, 5:6]
        nm24_c = cc_s[:, 6:7]
        n8003_c = cc_s[:, 7:8]
        bd2_0c = scl_s[:, 0:1]

        def bc(col, n):
            return col.broadcast_to((P, n))

        # ============ STAGE A: ray setup (batched over T) ============
        n24 = T * 3

        def st(shape, tag, dtype=dt.float32):
            return spool.tile(shape, dtype, tag=tag, name=tag)

        negd = st([P, T, 3], 's_negd')
        nc.vector.tensor_scalar(negd[:], rd_s[:], -1.0, None, Alu.mult)
        absd = st([P, T, 3], 's_absd')
        nc.vector.tensor_tensor(absd[:], rd_s[:], negd[:], Alu.max)
        dmask = st([P, T, 3], 's_dmask', dt.uint8)
        nc.vector.tensor_scalar(dmask[:], absd[:], 1e-9, None, Alu.is_lt)
        dsafe = st([P, T, 3], 's_dsafe')
        nc.vector.select(dsafe[:].rearrange("p t c -> p (t c)"),
                         dmask[:].rearrange("p t c -> p (t c)"),
                         bc(eps_c, n24),
                         rd_s[:].rearrange("p t c -> p (t c)"))
        invd = st([P, T, 3], 's_invd')
        nc.vector.reciprocal(invd[:], dsafe[:])
        a1 = st([P, T, 3], 's_a1')
        nc.vector.scalar_tensor_tensor(a1[:], ro_s[:], 1.0, invd[:],
                                       Alu.add, Alu.mult)
        b1 = st([P, T, 3], 's_b1')
        nc.vector.scalar_tensor_tensor(b1[:], ro_s[:], -1.0, invd[:],
                                       Alu.add, Alu.mult)
        mx = st([P, T, 3], 's_mx')
        nc.vector.tensor_tensor(mx[:], a1[:], b1[:], Alu.max)
        mn = st([P, T, 3], 's_mn')
        nc.vector.tensor_tensor(mn[:], a1[:], b1[:], Alu.min)
        tmin = st([P, T], 's_tmin')
        nc.vector.tensor_reduce(tmin[:], mx[:], AxX, Alu.min)
        tmax = st([P, T], 's_tmax')
        nc.vector.tensor_reduce(tmax[:], mn[:], AxX, Alu.max)
        near = st([P, T], 's_near')
        nc.vector.tensor_scalar(near[:], tmin[:], -1.0, MIN_NEAR,
                                Alu.mult, Alu.max)
        tmaxt = st([P, T], 's_tmaxt')
        nc.vector.tensor_scalar(tmaxt[:], tmax[:], -1.0, None, Alu.mult)
        fmask = st([P, T], 's_fmask', dt.uint8)
        nc.vector.tensor_tensor(fmask[:], tmaxt[:], near[:], Alu.is_lt)
        nearp = st([P, T], 's_nearp')
        nc.vector.tensor_scalar(nearp[:], near[:], 1e-2, None, Alu.add)
        far = st([P, T], 's_far')
        nc.vector.select(far[:], fmask[:], nearp[:], tmaxt[:])
        rng = st([P, T], 's_rng')
        nc.vector.tensor_tensor(rng[:], far[:], near[:], Alu.subtract)
        dzv = st([P, T], 's_dzv')
        nc.vector.tensor_scalar(dzv[:], rng[:], 1.0 / 63.0, None, Alu.mult)
        sdv = st([P, T], 's_sdv')
        nc.vector.tensor_scalar(sdv[:], rng[:], 1.0 / 64.0, None, Alu.mult)
        invdz = st([P, T], 's_invdz')
        nc.vector.reciprocal(invdz[:], dzv[:])
        inv2dz = st([P, T], 's_inv2dz')
        nc.vector.tensor_scalar(inv2dz[:], invdz[:], 2.0, None, Alu.mult)
        mid0 = st([P, T], 's_mid0')
        nc.vector.scalar_tensor_tensor(mid0[:], dzv[:], 0.5, near[:],
                                       Alu.mult, Alu.add)
        aoff = st([P, T], 's_aoff')
        nc.vector.tensor_tensor(aoff[:], near[:], dzv[:], Alu.subtract)

        # coarse z grid, batched: zc3[p,t,s] = near[p,t] + v128[s]*rng[p,t]
        zc3 = st([P, T, S], 's_zc3')
        v3 = v128_s[:].rearrange("p (o s) -> p o s", o=1).broadcast_to((P, T, S))
        rng3 = rng[:].rearrange("p (t o) -> p t o", o=1).broadcast_to((P, T, S))
        near3 = near[:].rearrange("p (t o) -> p t o", o=1).broadcast_to((P, T, S))
        nc.vector.tensor_tensor(zc3[:], v3, rng3, Alu.mult)
        nc.vector.tensor_tensor(zc3[:], zc3[:], near3, Alu.add)

        # persistent cross-phase arrays
        h20T = st([P, T, S], 's_h20T')
        Z3 = st([P, T, M], 's_Z3')
        dl3 = st([P, T, M], 's_dl3')

        def b3(col2, n):
            # [P, MB] -> [P, MB, n] broadcast
            return col2.rearrange("p (t o) -> p t o", o=1) \
                .broadcast_to((P, MB, n))

        # ================= machinery (batched per MB tiles) =================
        def mt(shape, tag, dtype=dt.float32):
            return mpool.tile(shape, dtype, tag=tag, name=tag)

        def do_mach(mb):
            t0 = mb * MB
            colM = slice(t0 * M, (t0 + MB) * M)
            colS = slice(t0 * S, (t0 + MB) * S)
            col62 = slice(t0 * 62, (t0 + MB) * 62)
            h20v = h20T[:, t0:t0 + MB, :]
            dz_sl = dzv[:, t0:t0 + MB]
            sd_sl = sdv[:, t0:t0 + MB]
            near_sl = near[:, t0:t0 + MB]
            i2dz_sl = inv2dz[:, t0:t0 + MB]
            mid0_sl = mid0[:, t0:t0 + MB]
            aoff_sl = aoff[:, t0:t0 + MB]

            TM2 = MB * M

            # --- coarse composite weights ---
            sig3 = mt([P, MB, S], 'm_sig3')
            nc.scalar.activation(sig3[:].rearrange("p t s -> p (t s)"),
                                 h20v.rearrange("p t s -> p (t s)"),
                                 Act.Exp, bias=bd2_0c)
            dsgc = mt([P, MB, S], 'm_dsgc')
            nc.vector.tensor_tensor(dsgc[:], sig3[:], b3(dz_sl, S), Alu.mult)
            nc.vector.tensor_tensor(dsgc[:, :, S - 1:S], sig3[:, :, S - 1:S],
                                    b3(sd_sl, 1), Alu.mult)
            emc = mt([P, MB, S], 'm_emc')
            nc.scalar.activation(emc[:].rearrange("p t s -> p (t s)"),
                                 dsgc[:].rearrange("p t s -> p (t s)"),
                                 Act.Exp, scale=-1.0)
            d0c = mt([P, MB, S], 'm_d0c')
            nc.vector.memset(d0c[:, :, 0:1], 0.0)
            nc.scalar.activation(d0c[:, :, 1:S], emc[:, :, 0:S - 1],
                                 Act.Identity, bias=e15_c)
            Tc = mt([P, MB, S], 'm_Tc')
            nc.vector.tensor_tensor_scan(
                Tc[:].rearrange("p t s -> p (t s)"),
                d0c[:].rearrange("p t s -> p (t s)"),
                oneS_s[:, colS], 0.0, Alu.mult, Alu.add)
            alpha = mt([P, MB, S], 'm_alpha')
            nc.scalar.activation(alpha[:].rearrange("p t s -> p (t s)"),
                                 emc[:].rearrange("p t s -> p (t s)"),
                                 Act.Identity, scale=-1.0, bias=ones_c)
            wts = mt([P, MB, S], 'm_wts')
            nc.vector.tensor_tensor(wts[:], alpha[:], Tc[:], Alu.mult)

            # --- pdf/cdf over weights[:,1:63] ---
            wp = mt([P, MB, 62], 'm_wp')
            nc.scalar.activation(wp[:], wts[:, :, 1:63], Act.Identity,
                                 bias=e5_c)
            ssum = mt([P, MB], 'm_ssum')
            nc.vector.tensor_reduce(ssum[:], wp[:], AxX, Alu.add)
            pinv = mt([P, MB], 'm_pinv')
            nc.vector.reciprocal(pinv[:], ssum[:])
            pdf = mt([P, MB, 62], 'm_pdf')
            nc.vector.tensor_tensor(pdf[:], wp[:], b3(pinv[:], 62), Alu.mult)
            cdf = mt([P, MB, 62], 'm_cdf')
            nc.vector.tensor_tensor_scan(
                cdf[:].rearrange("p t s -> p (t s)"), sm62_s[:, col62],
                pdf[:].rearrange("p t s -> p (t s)"), 0.0, Alu.mult, Alu.add)

            # --- scatter cdf onto per-segment 128-slot timelines ---
            r2 = mt([P, MB, 62], 'm_r2')
            r2f = r2[:].rearrange("p t s -> p (t s)")
            cdff = cdf[:].rearrange("p t s -> p (t s)")
            nc.scalar.activation(r2f, cdff, Act.Identity, scale=128.0,
                                 bias=m24_c)
            nc.scalar.activation(r2f, r2f, Act.Identity, bias=nm24_c)
            idx2f = mt([P, MB, 124], 'm_idx2f')
            i4 = idx2f[:].rearrange("p t (a b) -> p t a b", b=2)
            ev = i4[:, :, :, 0:1].rearrange("p t a b -> p t (a b)")
            od = i4[:, :, :, 1:2].rearrange("p t a b -> p t (a b)")
            nc.vector.tensor_tensor(
                ev, r2[:], iev_s[:, col62].rearrange("p (t s) -> p t s", t=MB),
                Alu.add)
            nc.scalar.activation(od, ev, Act.Identity, bias=ones_c)
            idx2i = mt([P, MB * 124], 'm_idx2i', dt.int16)
            nc.scalar.copy(idx2i[:], idx2f[:].rearrange("p t s -> p (t s)"))
            tlc2 = mt([P, MB * 256], 'm_tlc2', dt.int16)
            nc.gpsimd.local_scatter(tlc2[:], cdff.bitcast(dt.int16),
                                    idx2i[:], channels=P,
                                    num_elems=MB * 256, num_idxs=MB * 124)
            tlc = tlc2[:].bitcast(dt.float32)
            tlc3 = tlc.rearrange("p (t s) -> p t s", t=MB)

            # --- fills and counts on the timeline ---
            notC = mt([P, MB, M], 'm_notC')
            notCf = notC[:].rearrange("p t s -> p (t s)")
            nc.vector.tensor_scalar(notCf, tlc, 0.0, None, Alu.is_equal)
            notCp = mt([P, MB * M], 'm_notCp')
            nc.gpsimd.tensor_tensor(notCp[:], notCf, smM_s[:, colM], Alu.mult)
            kp1 = mt([P, MB, M], 'm_kp1')
            kp1f = kp1[:].rearrange("p t s -> p (t s)")
            nc.vector.tensor_tensor_scan(kp1f, smM_s[:, colM], notCf,
                                         0.0, Alu.mult, Alu.add)
            uu = mt([P, MB * M], 'm_uu')
            nc.scalar.activation(uu[:], kp1f, Act.Identity,
                                 scale=1.0 / 64.0, bias=nhalf_c)
            cntC = mt([P, MB * M], 'm_cntC')
            nc.vector.tensor_tensor(cntC[:], iop1_s[:, colM], kp1f,
                                    Alu.subtract)
            ffwd = mt([P, MB * M], 'm_ffwd')
            nc.vector.tensor_tensor_scan(ffwd[:], notCp[:], tlc, 0.0,
                                         Alu.mult, Alu.add)
            rnotC = mt([P, MB, M], 'm_rnotC')
            nc.scalar.copy(rnotC[:], notC[:, :, ::-1])
            rnotCp = mt([P, MB * M], 'm_rnotCp')
            nc.gpsimd.tensor_tensor(rnotCp[:],
                                    rnotC[:].rearrange("p t s -> p (t s)"),
                                    smM_s[:, colM], Alu.mult)
            rtlc = mt([P, MB, M], 'm_rtlc')
            nc.scalar.copy(rtlc[:], tlc3[:, :, ::-1])
            rbwd = mt([P, MB, M], 'm_rbwd')
            nc.vector.tensor_tensor_scan(
                rbwd[:].rearrange("p t s -> p (t s)"), rnotCp[:],
                rtlc[:].rearrange("p t s -> p (t s)"), 0.0, Alu.mult, Alu.add)

            # --- inverse-CDF lerp at u slots ---
            den = mt([P, MB, M], 'm_den')
            nc.vector.tensor_tensor(den[:], rbwd[:, :, ::-1],
                                    ffwd[:].rearrange("p (t s) -> p t s", t=MB),
                                    Alu.subtract)
            denf = den[:].rearrange("p t s -> p (t s)")
            mkd = mt([P, MB * M], 'm_mkd', dt.uint8)
            nc.vector.tensor_scalar(mkd[:], denf, 1e-5, None, Alu.is_lt)
            nc.vector.select(denf, mkd[:], bc(ones_c, TM2), denf)
            rden = mt([P, MB * M], 'm_rden')
            nc.vector.reciprocal(rden[:], denf)
            ttv = mt([P, MB * M], 'm_ttv')
            nc.vector.tensor_tensor(ttv[:], uu[:], ffwd[:], Alu.subtract)
            nc.vector.tensor_tensor(ttv[:], ttv[:], rden[:], Alu.mult)
            bg0 = mt([P, MB, M], 'm_bg0')
            nc.gpsimd.tensor_tensor(bg0[:],
                                    cntC[:].rearrange("p (t s) -> p t s", t=MB),
                                    b3(dz_sl, M), Alu.mult)
            nc.gpsimd.tensor_tensor(bg0[:], bg0[:], b3(mid0_sl, M), Alu.add)
            nz = mt([P, MB, M], 'm_nz')
            nc.vector.tensor_tensor(nz[:],
                                    ttv[:].rearrange("p (t s) -> p t s", t=MB),
                                    b3(dz_sl, M), Alu.mult)
            nc.vector.tensor_tensor(nz[:], nz[:], bg0[:], Alu.add)
            nzf = nz[:].rearrange("p t s -> p (t s)")

            # --- merge ranks into final (coarse ∪ fine) timeline ---
            q2 = mt([P, MB, M], 'm_q2')
            nc.vector.tensor_tensor(q2[:], nz[:], b3(near_sl, M), Alu.subtract)
            nc.vector.tensor_tensor(q2[:], q2[:], b3(i2dz_sl, M), Alu.mult)
            q2f = q2[:].rearrange("p t s -> p (t s)")
            nc.vector.tensor_scalar(q2f, q2f, 1.0, M24, Alu.add, Alu.add)
            nc.vector.tensor_scalar(q2f, q2f, M24, 0.0, Alu.subtract, Alu.max)
            nc.vector.tensor_scalar(q2f, q2f, 126.0, 8001.0, Alu.min, Alu.add)
            tk2 = mt([P, MB * M], 'm_tk2')
            nc.scalar.activation(tk2[:], kp1f, Act.Identity, scale=2.0,
                                 bias=n8003_c)
            mk2 = mt([P, MB * M], 'm_mk2')
            nc.vector.tensor_scalar(mk2[:], kp1f, 64.5, None, Alu.is_gt)
            minv = mt([P, MB * M], 'm_minv')
            nc.vector.scalar_tensor_tensor(minv[:], mk2[:], 1.0, notCf,
                                           Alu.add, Alu.subtract)
            m2 = mt([P, MB * M], 'm_m2')
            nc.vector.scalar_tensor_tensor(m2[:], minv[:], -4000.0, q2f,
                                           Alu.mult, Alu.add)
            ms = mt([P, MB * M], 'm_ms')
            nc.vector.tensor_tensor_scan(ms[:], smM_s[:, colM], m2[:],
                                         0.0, Alu.mult, Alu.max)
            rkv = mt([P, MB * M], 'm_rkv')
            nc.vector.tensor_tensor(rkv[:], tk2[:], ms[:], Alu.add)
            nc.vector.tensor_scalar(rkv[:], rkv[:], 254.0, None, Alu.min)
            nc.vector.scalar_tensor_tensor(rkv[:], minv[:], -4000.0, rkv[:],
                                           Alu.mult, Alu.add)
            fidx2f = mt([P, MB, 256], 'm_fidx2f')
            f4 = fidx2f[:].rearrange("p t (a b) -> p t a b", b=2)
            fev = f4[:, :, :, 0:1].rearrange("p t a b -> p t (a b)")
            fod = f4[:, :, :, 1:2].rearrange("p t a b -> p t (a b)")
            nc.vector.tensor_tensor(
                fev, rkv[:].rearrange("p (t s) -> p t s", t=MB),
                s256_s[:, colM].rearrange("p (t s) -> p t s", t=MB), Alu.add)
            nc.scalar.activation(fod, fev, Act.Identity, bias=ones_c)
            fidx2i = mt([P, MB * 256], 'm_fidx2i', dt.int16)
            nc.scalar.copy(fidx2i[:], fidx2f[:].rearrange("p t s -> p (t s)"))
            zf2 = mt([P, MB * 256], 'm_zf2', dt.int16)
            nc.gpsimd.local_scatter(zf2[:], nzf.bitcast(dt.int16),
                                    fidx2i[:], channels=P,
                                    num_elems=MB * 256, num_idxs=MB * 256)
            zsc = zf2[:].bitcast(dt.float32)

            # --- fill coarse slots with uniform grid ---
            isCC = mt([P, MB, M], 'm_isCC')
            isCCf = isCC[:].rearrange("p t s -> p (t s)")
            nc.vector.tensor_scalar(isCCf, zsc, 0.0, None, Alu.is_equal)
            cum2 = mt([P, MB, M], 'm_cum2')
            nc.vector.tensor_tensor_scan(
                cum2[:].rearrange("p t s -> p (t s)"), smM_s[:, colM],
                isCCf, 0.0, Alu.mult, Alu.add)
            zcf = mt([P, MB, M], 'm_zcf')
            nc.gpsimd.tensor_tensor(zcf[:], cum2[:], b3(dz_sl, M), Alu.mult)
            nc.gpsimd.tensor_tensor(zcf[:], zcf[:], b3(aoff_sl, M), Alu.add)
            Zv = Z3[:, t0:t0 + MB, :]
            nc.vector.tensor_tensor(Zv, isCC[:], zcf[:], Alu.mult)
            nc.vector.tensor_tensor(
                Zv, Zv, zsc.rearrange("p (t s) -> p t s", t=MB), Alu.add)
            dv = dl3[:, t0:t0 + MB, :]
            nc.vector.tensor_tensor(dv[:, :, 0:M - 1], Zv[:, :, 1:M],
                                    Zv[:, :, 0:M - 1], Alu.subtract)
            nc.scalar.copy(dv[:, :, M - 1:M],
                           sd_sl.rearrange("p (t o) -> p t o", o=1))


        # ================= PHASE 1: coarse MLP + sigma =================
        for t in range(T):
            xyzc = wpool.tile([P, 3, S], dt.float32r, tag="xyzc", name="xyzc")
            for c in range(3):
                nc.vector.scalar_tensor_tensor(
                    xyzc[:, c, :], zc3[:, t, :], rd_s[:, t, c:c + 1],
                    bc(ro_s[:, t, c:c + 1], S), Alu.mult, Alu.add)
            scr = dpool.tile([3, P, S], dt.float32r, tag="xyzscr", name="xyzscr")
            nc.sync.dma_start(scr[:].rearrange("c p s -> p c s"), xyzc[:])
            rhs6 = rpool.tile([6, P * S // 2], dt.float32r, tag="rhs6c",
                              name="rhs6c")
            scrf = scr[:].rearrange("c p s -> c (p s)")
            half = P * S // 2
            nc.sync.dma_start(rhs6[0:3, :], scrf[:, 0:half])
            nc.sync.dma_start(rhs6[3:6, :], scrf[:, half:2 * half])

            sgs = dpool.tile([2, 64, S], dt.float32, tag="sigscr", name="sigscr")
            sgsf = sgs[:].rearrange("h p s -> (h p s)")

            pS = None
            for hf in range(4):          # 4 half-groups of 1024 cols
                pA = pp1.tile([128, 1024], dt.float32, tag="pA", name="pA")
                for c2 in range(2):
                    mmr(pA[:, 512 * c2:512 * (c2 + 1)], lhsT6_s[:],
                        rhs6[:, 1024 * hf + 512 * c2:1024 * hf + 512 * (c2 + 1)],
                        start=True, stop=True)
                rh1 = bpool.tile([128, 1024], dt.bfloat16, tag="rh1", name="rh1")
                if hf % 2 == 0:
                    nc.vector.tensor_scalar(rh1[:], pA[:], b1_s[:], 0.0,
                                            Alu.add, Alu.max)
                else:
                    nc.scalar.activation(rh1[:], pA[:], Act.Relu, bias=b1_s[:])
                if hf % 2 == 0:
                    pS = pp1.tile([128, 512], dt.float32, tag="pS", name="pS")
                    if SIM_SAFE:
                        nc.vector.memset(pS[:], 0.0)
                for c2 in range(2):
                    cc_g = 2 * (hf % 2) + c2
                    pos = 32 * cc_g
                    nc.tensor.matmul(pS[pos:pos + 2, :], w0p_s[:],
                                     rh1[:, 512 * c2:512 * (c2 + 1)],
                                     start=True, stop=True,
                                     tile_position=(0, pos))
                if hf % 2 == 1:
                    ps_i = hf // 2
                    sg_sb = bpool.tile([128, 512], dt.float32, tag="sgsb",
                                       name="sgsb")
                    if ps_i == 0:
                        nc.vector.tensor_copy(sg_sb[:], pS[:])
                    else:
                        nc.scalar.copy(sg_sb[:], pS[:])
                    # rows (32a+h) -> dram (h, p=32*ps+8a+q, s)
                    for a_ in range(4):
                        dst = sgs[:, 32 * ps_i + 8 * a_:
                                  32 * ps_i + 8 * (a_ + 1), :] \
                            .rearrange("h p s -> h (p s)")
                        nc.sync.dma_start(dst, sg_sb[32 * a_:32 * a_ + 2, :])
            nc.sync.dma_start(h20T[:, t, :],
                              sgs[:].rearrange("h p s -> (h p) s"))
            if t % MB == MB - 1:
                do_mach(t // MB)

        # ================= PHASE 2: fine MLP + composite =================
        fh = P * M // 2
        for t in range(T):
            xyzm = wpool.tile([P, 3, M], dt.float32r, tag="xyzm", name="xyzm")
            for c in range(3):
                nc.vector.scalar_tensor_tensor(
                    xyzm[:, c, :], Z3[:, t, :], rd_s[:, t, c:c + 1],
                    bc(ro_s[:, t, c:c + 1], M), Alu.mult, Alu.add)
            scr2 = dpool.tile([3, P, M], dt.float32r, tag="xyzscr2",
                              name="xyzscr2")
            nc.sync.dma_start(scr2[:].rearrange("c p s -> p c s"), xyzm[:])
            dT_sb = wpool.tile([4, P], dt.float32, tag="dTsb", name="dTsb")
            nc.sync.dma_start(dT_sb[:], dT_in[t])
            rhs6f = rpool.tile([6, fh], dt.float32r, tag="rhs6f", name="rhs6f")
            scr2f = scr2[:].rearrange("c p s -> c (p s)")
            nc.sync.dma_start(rhs6f[0:3, :], scr2f[:, 0:fh])
            nc.sync.dma_start(rhs6f[3:6, :], scr2f[:, fh:2 * fh])

            # dterm pairs [128, 64] (v1 form)
            pD = pp1.tile([128, 512], dt.float32, tag="pS", name="pS")
            if SIM_SAFE:
                nc.vector.memset(pD[:], 0.0)
            nc.tensor.matmul(pD[0:64, 0:64], dlhs_s[:], dT_sb[:, 0:64],
                             start=True, stop=True)
            nc.tensor.matmul(pD[64:128, 0:64], dlhs_s[:], dT_sb[:, 64:128],
                             start=True, stop=True, tile_position=(0, 64))
            dtm = bpool.tile([128, 64], dt.float32, tag="dtm", name="dtm")
            nc.scalar.copy(dtm[:], pD[:, 0:64])

            sgs2 = dpool.tile([2, 64, M], dt.float32, tag="sigscr2",
                              name="sigscr2")
            sgs2f = sgs2[:].rearrange("h p s -> (h p s)")
            rgbs = dpool.tile([2, 3, 64, M], dt.float32, tag="rgbscr",
                              name="rgbscr")
            rgbsw = rgbs[:].rearrange("h c p s -> (h c) (p s)")

            for g in range(4):
                rgb_sb = bpool.tile([6, 2048], dt.float32, tag="rgbsb",
                                    name="rgbsb")
                pS = pp1.tile([128, 512], dt.float32, tag="pS", name="pS")
                if SIM_SAFE:
                    nc.vector.memset(pS[:], 0.0)
                for hf in range(2):
                    base = g * 2048 + hf * 1024
                    pA = pp1.tile([128, 1024], dt.float32, tag="pA", name="pA")
                    for c2 in range(2):
                        mmr(pA[:, 512 * c2:512 * (c2 + 1)], lhsT6_s[:],
                            rhs6f[:, base + 512 * c2:base + 512 * (c2 + 1)],
                            start=True, stop=True)
                    rh1f = bpool.tile([128, 1024], dt.bfloat16, tag="rh1",
                                      name="rh1")
                    if hf == 0:
                        nc.vector.tensor_scalar(rh1f[:], pA[:], b1_s[:], 0.0,
                                                Alu.add, Alu.max)
                    else:
                        nc.scalar.activation(rh1f[:], pA[:], Act.Relu,
                                             bias=b1_s[:])
                    for c2 in range(2):
                        cc4 = 2 * hf + c2
                        pos = 32 * cc4
                        nc.tensor.matmul(pS[pos:pos + 2, :], w0p_s[:],
                                         rh1f[:, 512 * c2:512 * (c2 + 1)],
                                         start=True, stop=True,
                                         tile_position=(0, pos))
                    pG = pp2.tile([128, 1024], dt.float32, tag="pG", name="pG")
                    for c2 in range(2):
                        nc.tensor.matmul(pG[:, 512 * c2:512 * (c2 + 1)],
                                         wgc_s[:],
                                         rh1f[:, 512 * c2:512 * (c2 + 1)],
                                         start=True, stop=True)
                    ch1 = bpool.tile([128, 1024], dt.float32r, tag="ch1",
                                     name="ch1")
                    # fused relu(pG + dterm): dterm is constant per
                    # 128-col ray-pair block -> per-block bias column
                    for a_ in range(8):
                        blk = slice(128 * a_, 128 * (a_ + 1))
                        dcol = dtm[:, g * 16 + hf * 8 + a_:
                                   g * 16 + hf * 8 + a_ + 1]
                        if a_ % 2 == 0:
                            nc.scalar.activation(ch1[:, blk], pG[:, blk],
                                                 Act.Relu, bias=dcol)
                        else:
                            nc.vector.tensor_scalar(ch1[:, blk], pG[:, blk],
                                                    dcol, 0.0, Alu.add,
                                                    Alu.max)
                    for c2 in range(2):
                        pC = pp1.tile([6, 512], dt.float32, tag="pC", name="pC")
                        mmr(pC[:], wc2_s[:], ch1[:, 512 * c2:512 * (c2 + 1)],
                            start=True, stop=True)
                        osl = rgb_sb[:, 1024 * hf + 512 * c2:
                                     1024 * hf + 512 * (c2 + 1)]
                        if (2 * hf + c2) % 2 == 0:
                            nc.scalar.activation(osl, pC[:], Act.Identity,
                                                 bias=bc2_s[:])
                        else:
                            nc.vector.tensor_scalar(osl, pC[:], bc2_s[:],
                                                    None, Alu.add)
                # sigma eviction (packed) + DMA
                sg_sb = bpool.tile([128, 512], dt.float32, tag="sgsb",
                                   name="sgsb")
                if g % 2 == 0:
                    nc.vector.tensor_copy(sg_sb[:], pS[:])
                else:
                    nc.scalar.copy(sg_sb[:], pS[:])
                for a_ in range(4):
                    dst = sgs2[:, 16 * g + 4 * a_:16 * g + 4 * (a_ + 1), :] \
                        .rearrange("h p s -> h (p s)")
                    nc.sync.dma_start(dst, sg_sb[32 * a_:32 * a_ + 2, :])
                nc.sync.dma_start(rgbsw[:, g * 2048:(g + 1) * 2048], rgb_sb[:])

            # ---- composite in rays layout ----
            def wt(shape, tag, dtype=dt.float32):
                return wpool.tile(shape, dtype, tag=tag, name=tag)

            h20m = wt([P, M], "h20m")
            nc.sync.dma_start(h20m[:], sgs2[:].rearrange("h p s -> (h p) s"))
            sigm = wt([P, M], "sigm")
            nc.scalar.activation(sigm[:], h20m[:], Act.Exp, bias=bd2_0c)
            dsg2 = wt([P, M], "dsg2")
            nc.vector.tensor_tensor(dsg2[:], dl3[:, t, :], sigm[:], Alu.mult)
            em2 = wt([P, M], "em2")
            nc.scalar.activation(em2[:], dsg2[:], Act.Exp, scale=-1.0)
            sb2 = wt([P, M], "sb2")
            nc.vector.memset(sb2[:, 0:1], 1.0)
            nc.scalar.activation(sb2[:, 1:M], em2[:, 0:M - 1],
                                 Act.Identity, bias=e15_c)
            Tm = wt([P, M], "Tm")
            nc.vector.tensor_tensor_scan(Tm[:], sb2[:], zeroM_s[:], 1.0,
                                         Alu.mult, Alu.add)
            alpm = wt([P, M], "alpm")
            nc.scalar.activation(alpm[:], em2[:], Act.Identity,
                                 scale=-1.0, bias=ones_c)
            wm = wt([P, M], "wm")
            nc.vector.tensor_tensor(wm[:], alpm[:], Tm[:], Alu.mult)
            wsum = wt([P, 1], "wsum")
            nc.vector.tensor_reduce(wsum[:], wm[:], AxX, Alu.add)
            wmm = wt([P, M], "wmm")
            nc.vector.scalar_tensor_tensor(wmm[:], wm[:], 1e-4, wm[:],
                                           Alu.is_gt, Alu.mult)

            rgbp = wt([P, 3, M], "rgbp")
            for h_ in range(2):
                nc.sync.dma_start(rgbp[64 * h_:64 * (h_ + 1), :, :],
                                  rgbs[h_].rearrange("c p s -> p c s"))
            erg = wt([P, 3, M], "erg")
            nc.scalar.activation(erg[:].rearrange("p c s -> p (c s)"),
                                 rgbp[:].rearrange("p c s -> p (c s)"),
                                 Act.Exp, scale=-1.0)
            nc.scalar.activation(erg[:].rearrange("p c s -> p (c s)"),
                                 erg[:].rearrange("p c s -> p (c s)"),
                                 Act.Identity, bias=ones_c)
            rgbv = wt([P, 3, M], "rgbv")
            nc.vector.reciprocal(rgbv[:], erg[:])
            nc.vector.tensor_tensor(
                rgbv[:], rgbv[:],
                wmm[:].rearrange("p (o s) -> p o s", o=1)
                .broadcast_to((P, 3, M)), Alu.mult)
            img = wt([P, 3], "img")
            nc.vector.tensor_reduce(img[:], rgbv[:], AxX, Alu.add)
            bgw = wt([P, 1], "bgw")
            nc.vector.tensor_scalar(bgw[:], wsum[:], -1.0, 1.0, Alu.mult,
                                    Alu.add)
            nc.vector.scalar_tensor_tensor(img[:], bg_s[:], bgw[:], img[:],
                                           Alu.mult, Alu.add)
            nc.sync.dma_start(img_out[:, t, :], img[:])

    nc.compile()
    return nc


def _host_constants(inputs):
    Wd1 = np.asarray(inputs["Wd1"], np.float32)
    bd1 = np.asarray(inputs["bd1"], np.float32)
    Wd2 = np.asarray(inputs["Wd2"], np.float32)
    bd2 = np.asarray(inputs["bd2"], np.float32)
    Wc1 = np.asarray(inputs["Wc1"], np.float32)
    bc1 = np.asarray(inputs["bc1"], np.float32)
    Wc2 = np.asarray(inputs["Wc2"], np.float32)
    bc2 = np.asarray(inputs["bc2"], np.float32)
    tval = float(np.asarray(inputs["time"]).reshape(()))

    W1 = Wd1[:3]
    b1p = bd1 + tval * Wd1[3]
    w0 = Wd2[:, 0:1]
    Wgc = (Wd2[:, 1:].astype(np.float64) @ Wc1[3:].astype(np.float64)) \
        .astype(np.float32)
    bgc = (bd2[1:].astype(np.float64) @ Wc1[3:].astype(np.float64)) \
        .astype(np.float32)
    bd2_0 = float(bd2[0])

    lhsT6 = np.zeros((6, 128), np.float32)
    lhsT6[0:3, 0:64] = W1
    lhsT6[3:6, 64:128] = W1
    b1col = np.concatenate([b1p, b1p]).reshape(128, 1).astype(np.float32)

    w0pair = np.zeros((128, 2), np.float32)
    w0pair[0:64, 0:1] = w0
    w0pair[64:128, 1:2] = w0

    wgcpair = np.zeros((128, 128), np.float32)
    wgcpair[0:64, 0:64] = Wgc
    wgcpair[64:128, 64:128] = Wgc

    wc2pair = np.zeros((128, 6), np.float32)
    wc2pair[0:64, 0:3] = Wc2
    wc2pair[64:128, 3:6] = Wc2

    dlhs = np.zeros((4, 64), np.float32)
    dlhs[0:3] = Wc1[:3]
    dlhs[3] = bc1 + bgc

    bc2col = np.concatenate([bc2, bc2]).reshape(6, 1).astype(np.float32)


    v = np.linspace(0.0, 1.0, S, dtype=np.float32)

    jM = np.arange(T * M)
    j62 = np.arange(T * 62)
    jS = np.arange(T * S)
    segmaskM = (jM % M != 0).astype(np.float32)
    iop1T = (jM % M + 1).astype(np.float32)
    seg256T = (256 * ((jM // M) % MB)).astype(np.float32)
    segmask62 = (j62 % 62 != 0).astype(np.float32)
    iev62T = (2 * (j62 % 62) + 256 * ((j62 // 62) % MB)).astype(np.float32)
    oneSst = (jS % S == 0).astype(np.float32)

    def rep(row):
        return np.broadcast_to(row, (P,) + row.shape).copy()

    return {
        "v128": rep(v),
        "cc": rep(np.array([1.0, 1e-9, 1e-15, 1e-5, -1.0 / 128.0,
                    16777216.0, -16777216.0, -8003.0],
                   np.float32)),
        "segmaskM": rep(segmaskM),
        "iop1T": rep(iop1T),
        "seg256T": rep(seg256T),
        "segmask62": rep(segmask62),
        "iev62T": rep(iev62T),
        "oneSst": rep(oneSst),
        "zeroM": np.zeros((P, M), np.float32),
        "lhsT6": lhsT6, "b1col": b1col,
        "w0pair": w0pair.astype(ml_dtypes.bfloat16),
        "wgcpair": wgcpair.astype(ml_dtypes.bfloat16),
        "wc2pair": wc2pair, "dlhs": dlhs,
        "bc2col": bc2col,
        "bgrep": np.broadcast_to(
            np.asarray(inputs["background_color"], np.float32), (P, 3)).copy(),
        "scl": np.broadcast_to(
            np.array([bd2_0, 0, 0, 0], np.float32), (P, 4)).copy(),
    }


def kernel(**inputs):
    global _BUILT
    assert int(inputs["num_steps"]) == S
    assert int(inputs["upsample_steps"]) == U

    if _BUILT is None:
        _BUILT = _build()
    nc = _BUILT

    consts = _host_constants(inputs)
    ro = np.asarray(inputs["rays_o"], np.float32).reshape(NRAYS, 3)
    rd = np.asarray(inputs["rays_d"], np.float32).reshape(NRAYS, 3)

    in_maps = []
    for c in range(NCORES):
        sl_o = ro[c * R:(c + 1) * R].reshape(T, P, 3)
        sl_d = rd[c * R:(c + 1) * R].reshape(T, P, 3)
        dT = np.ones((T, 4, P), np.float32)
        dT[:, 0:3, :] = sl_d.transpose(0, 2, 1)
        m = {
            "rays_o_k": np.ascontiguousarray(sl_o.transpose(1, 0, 2)),
            "rays_d_k": np.ascontiguousarray(sl_d.transpose(1, 0, 2)),
            "dT_k": dT,
        }
        m.update(consts)
        in_maps.append(m)

    res = run_bass_kernel_spmd(nc, in_maps, core_ids=list(range(NCORES)))
    global LAST_RESULT
    LAST_RESULT = res
    outs = []
    for c in range(NCORES):
        img = res.results[c]["img_k"]
        outs.append(img.transpose(1, 0, 2).reshape(R, 3))
    return np.concatenate(outs, 0).reshape(1, NRAYS, 3)
